# revision 28
# baseline (speedup 1.0000x reference)
"""Trainium2 Bass kernel for nn_Attention_5815385719367 (gnn_message_passing).

Computation (see reference):
  map_q/k/v = map_code @ Wq/Wk/Wv ; obs_k/v = obs_code @ Wk/Wv
  scores    = [sum(q*k,-1) | q @ obs_k.T] / 8
  w         = softmax(scores)
  agg       = w[:, :1]*glu(map_v) + w[:, 1:] @ glu(obs_v)
  out       = LN(agg @ Wo + bo + map_code) * gamma + beta

Sharding: data-parallel over N_map rows (2048 rows/core x 8 cores);
obs_code and weights replicated. No collectives.

v2 design notes (per core):
  - scores computed TRANSPOSED in PSUM: ST[obs=128, map] via PE ROW-TILED
    pairs: even obs block's k.T sits on SBUF partitions 0-63 (PE tile
    (0,0)), odd block's on partitions 64-127 (tile (64,0)); the two
    256-cycle streams run CONCURRENTLY in different PE row groups, so
    ST costs ~0.5 cyc/col. qT and okT are produced pre-duplicated /
    parity-split by projecting with host-duplicated weights
    ([64,128] wq|wq and wk|wk), so no cross-partition copies exist.
  - softmax exp is the hard wall (1 elem/lane/cycle on ACT): split it
    between ACT (direct exp -> fp8e4m3, logits shifted by -1 so
    exp <= ~110 < 240 = trn-e4m3 max) and DVE (Schraudolph: one
    mult-add tensor_scalar into uint8, whose bit pattern IS the
    e4m3 log-domain approximation; errors ~+-5% are noise-like and
    average out over 8k obs).
  - PV runs fp8 DoubleRow over block PAIRS: stationary
    gob8[128, 2, 66] = glu(obs_v)|ones|zero-pad for (even, odd)
    blocks, moving pt8[128, 2, 512] = exp'd scores; contraction is
    256 at 0.5 cyc/col. M padded 33->66 keeps col tiling off
    (DoubleRow is incompatible with column tiling). The ones column
    accumulates the softmax denominator for free.
  - self-attention term folded in after Wo (as v1): agg row 32 is
    seeded with selfexp, epilogue adds selfexp*(glu(map_v)@Wo) and
    divides by the denominator; the -1 logit shift cancels exactly.
  - projections in bf16 (inputs shipped bf16), epilogue Wo in bf16.
  - GPSIMD does SBUF-side elementwise work (sigmoid affine steps,
    map+bo, gamma/beta) since it cannot touch PSUM.
"""

import numpy as np

import concourse.bass as bass
import concourse.bacc as bacc
import concourse.tile as tile
from concourse import mybir
from concourse.bass_utils import run_bass_kernel_spmd

NCORES = 8
NM, NO, E = 16384, 8192, 64
NS = NM // NCORES            # 2048 map rows per core
H = E // 2                   # 32
TEMP = 8.0
EPS = 1e-6
P = 128
NT = NS // P                 # 16 row tiles per core
GW = 512                     # map group width (psum bank)
NPAIR = NO // 256            # 32 obs block-pairs
SHIFT = -2.0                 # logit shift: exp(l - SHIFT), cancels in ratio.
                             # Logits empirically span [-8.1, 8.32]; e5m2's
                             # 22-e-fold range with shift -2 covers all of it
                             # (max exp ~ e^10.3 = 3e4 < 57344) with no
                             # clipping at either end.
MPAD = 48                    # padded PV output partitions (33 real; %16 for DoubleRow ldweights step)

F32 = mybir.dt.float32
F32R = mybir.dt.float32r
BF16 = mybir.dt.bfloat16
FP8 = mybir.dt.float8e4
FP8E5 = mybir.dt.float8e5
U8 = mybir.dt.uint8
AF = mybir.ActivationFunctionType
ALU = mybir.AluOpType
DR = mybir.MatmulPerfMode.DoubleRow

# Schraudolph constants for uint8 e5m2 log-domain exp of RAW score s:
#   i = 4*log2(exp(s/8 - SHIFT)) + 60 - sawtooth_center
# DVE float->uint8 conversion saturates [0,255] and rounds RNE (probed).
SCH_A = 4.0 * 1.4426950408889634 / TEMP   # 0.72135
SCH_B = 60.0 - 4.0 * 1.4426950408889634 * SHIFT - 0.229

# layout of the bf16 weight pack [64, BW]
_WQ0 = 0              # wq duplicated [64, 128]
_WK0 = 128            # wk duplicated [64, 128]
_WV0 = 256            # wv [64, 64]
BW = 320

# layout of the f32r pack [64, FW]
_ONES0 = 0            # ones column [64, 1]
_SEL0 = 1             # sel row at partition 32: [1, 66]
_WOE0 = 1 + MPAD      # woe [33 rows used, 66]
FW = 1 + MPAD + E + 2


def _bc_part(ap, n):
    """Broadcast a [x, ...] AP along a new leading partition dim of n."""
    return bass.AP(tensor=ap.tensor, offset=ap.offset, ap=[[0, n]] + list(ap.ap))


def _emit(tc, out_d, map_rows_d, mapT_d, obsT_d, wpb_d, wpf_d, c8_d, vec_d,
          dbg=None, exp_act_frac=0.5):
    nc = tc.nc
    with tc.tile_pool(name="consts", bufs=1) as consts, \
         tc.tile_pool(name="big", bufs=1) as big, \
         tc.tile_pool(name="sb_sm", bufs=3) as sb_sm, \
         tc.tile_pool(name="sb_pt", bufs=4) as sb_pt, \
         tc.tile_pool(name="ps", bufs=3, space="PSUM") as ps, \
         tc.tile_pool(name="ps_agg", bufs=2, space="PSUM") as ps_agg:

        # ---------------- constants ----------------
        wpb = consts.tile([E, BW], BF16)          # bf16 weights pack
        nc.sync.dma_start(wpb, wpb_d)
        wq2 = wpb[:, _WQ0:_WQ0 + 128]             # [64,128] wq|wq
        wk2 = wpb[:, _WK0:_WK0 + 128]             # [64,128] wk|wk
        wv = wpb[:, _WV0:_WV0 + E]                # [64,64]

        wpf = consts.tile([E, FW], F32R)
        nc.sync.dma_start(wpf, wpf_d)
        ones64 = wpf[:, _ONES0:_ONES0 + 1]
        sel66 = wpf[H:H + 1, _SEL0:_SEL0 + MPAD]  # row at partition 32
        woe = wpf[0:H + 1, _WOE0:_WOE0 + E + 2]   # [33,66]

        vecs = consts.tile([P, 3 * E + 1], F32)   # bo|gamma|beta|-shift
        nc.sync.dma_start(vecs, _bc_part(vec_d, P))
        bo_b = vecs[:, 0:E]
        ga_b = vecs[:, E:2 * E]
        be_b = vecs[:, 2 * E:3 * E]
        msh = vecs[:, 3 * E:3 * E + 1]            # -SHIFT bias column

        # ---------------- big arenas + input DMAs ----------------
        mapT = big.tile([E, NS], BF16)
        obsT = big.tile([E, NO], BF16)
        for lo, hi, t_, s_ in ((0, 512, mapT, mapT_d),
                               (0, 1024, obsT, obsT_d),
                               (512, 1024, mapT, mapT_d),
                               (1024, 2048, obsT, obsT_d),
                               (1024, 2048, mapT, mapT_d),
                               (2048, 4096, obsT, obsT_d),
                               (4096, 8192, obsT, obsT_d)):
            nc.sync.dma_start(t_[:, lo:hi], s_[:, lo:hi])
        map_rows = big.tile([P, NT, E], F32)
        nc.sync.dma_start(map_rows, map_rows_d.rearrange("(t p) e -> p t e", p=P))

        qT = big.tile([P, NS], F32R)              # map_q.T duplicated halves
        gmT = big.tile([H + 1, NS], F32R)         # [glu(map_v).T ; selfexp]
        okT = big.tile([P, NPAIR, P], F32R)       # obs_k.T parity-split
        gob8 = big.tile([P, NPAIR, 2, MPAD], FP8)  # glu(obs_v)|1|0 pairs
        ags = big.tile([H + 1, NS], F32R)         # [numer.T ; denom]
        map_pb = big.tile([P, NT, E], F32)        # map + bo
        out_pre = big.tile([P, NT, E], F32)
        out_all = big.tile([P, NT, E], F32)
        mvC = big.tile([P, NT, 2], F32)
        rstd = big.tile([P, NT], F32)

        # gob8 static columns: ones at h=32, zeros at h=33..65 (DMA from
        # the small HBM consts tensor, replicated via zero strides)
        gob8f = gob8.rearrange("p a b c -> p (a b) c")
        C8W = MPAD - H
        ones_src = bass.AP(tensor=c8_d.tensor, offset=c8_d.offset,
                           ap=[[C8W, P], [0, 2 * NPAIR], [0, 1]])
        nc.sync.dma_start(gob8f[:, :, H:H + 1], ones_src)
        zero_src = bass.AP(tensor=c8_d.tensor, offset=c8_d.offset + 1,
                           ap=[[C8W, P], [0, 2 * NPAIR], [1, MPAD - H - 1]])
        nc.sync.dma_start(gob8f[:, :, H + 1:MPAD], zero_src)

        # map + bo on gpsimd (all-SBUF)
        bo_rep = bass.AP(tensor=bo_b.tensor, offset=bo_b.offset,
                         ap=[list(bo_b.ap[0]), [0, NT], [1, E]])
        nc.gpsimd.tensor_tensor(out=map_pb, in0=map_rows, in1=bo_rep,
                                op=ALU.add)

        # ---------------- prologue pieces ----------------
        def map_chunk(c):
            """q (duplicated), selfexp, glu(map_v) for map cols [c*512, ..)."""
            sl = slice(c * GW, (c + 1) * GW)
            q_ps = ps.tile([P, 2, GW], F32, tag="st", name=f"qps{c}")
            nc.tensor.matmul(q_ps[:, 0, :], wq2, mapT[:, sl],
                             start=True, stop=True)
            nc.vector.tensor_copy(qT[:, sl], q_ps[:, 0, :])
            k_ps = ps.tile([P, 2, GW], F32, tag="st", name=f"kps{c}")
            nc.tensor.matmul(k_ps[:, 0, :], wk2, mapT[:, sl],
                             start=True, stop=True)
            qk = sb_sm.tile([E, GW], F32R, tag="qk", name=f"qk{c}")
            nc.vector.tensor_tensor(out=qk, in0=qT[0:E, sl],
                                    in1=k_ps[0:E, 0, :], op=ALU.mult)
            # self-score sum lands in the unused upper half of k_ps
            ss_ps = k_ps[0:1, 1, :]
            nc.tensor.matmul(ss_ps, ones64, qk, start=True, stop=True)
            nc.scalar.activation(gmT[H:H + 1, sl], ss_ps, AF.Exp,
                                 scale=1.0 / TEMP, bias=msh[0:1])
            v_ps = ps.tile([P, 2, GW], F32, tag="st", name=f"vps{c}")
            nc.tensor.matmul(v_ps[0:E, 0, :], wv, mapT[:, sl],
                             start=True, stop=True)
            th = sb_sm.tile([H, GW], F32, tag="th", name=f"th{c}")
            nc.scalar.activation(th, v_ps[H:E, 0, :], AF.Tanh, scale=0.5)
            nc.gpsimd.tensor_scalar(out=th, in0=th, scalar1=0.5, scalar2=0.5,
                                    op0=ALU.mult, op1=ALU.add)
            nc.vector.tensor_tensor(out=gmT[0:H, sl], in0=v_ps[0:H, 0, :],
                                    in1=th, op=ALU.mult)

        def obs_k_chunk2(c2, eng="v"):
            """okT parity-split fill for TWO obs chunks (one PSUM alloc).

            chunk c covers obs cols [c*512, ..) = blocks 4c..4c+3; even
            blocks land on partitions 0-63 of okT, odd blocks on 64-127
            (via the duplicated upper half of the wk2 projection, so no
            cross-partition movement is needed)."""
            k_ps = ps.tile([P, 2, GW], F32, tag="st", name=f"okps{c2}")
            for t in range(2):
                c = 2 * c2 + t
                sl = slice(c * GW, (c + 1) * GW)
                nc.tensor.matmul(k_ps[:, t, :], wk2, obsT[:, sl],
                                 start=True, stop=True)
            for t in range(2):
                c = 2 * c2 + t
                ev_in = k_ps[0:E, t, :].rearrange("p (b m) -> p b m",
                                                  b=2)[:, :, 0:P]
                ev_out = okT[0:E, 2 * c:2 * c + 2, :]
                od_in = k_ps[E:P, t, :].rearrange("p (b m) -> p b m",
                                                  b=2)[:, :, P:2 * P]
                od_out = okT[E:P, 2 * c:2 * c + 2, :]
                if eng == "v":
                    nc.vector.tensor_copy(ev_out, ev_in)
                    nc.vector.tensor_copy(od_out, od_in)
                else:
                    nc.scalar.copy(ev_out, ev_in)
                    nc.scalar.copy(od_out, od_in)

        def obs_v_batch2(c2, nb=16):
            """glu(obs_v) for nb consecutive obs blocks (one PSUM alloc)."""
            v_ps = ps.tile([P, 16, E], F32, tag="st", name=f"ovps{c2}")
            for b in range(nb):
                blk = c2 * 16 + b
                nc.tensor.matmul(v_ps[:, b, :],
                                 obsT[:, blk * P:(blk + 1) * P], wv,
                                 start=True, stop=True)
            tho = sb_sm.tile([P, 16, H], F32, tag="tho", name=f"tho{c2}")
            nc.scalar.activation(tho[:, 0:nb, :], v_ps[:, 0:nb, H:E],
                                 AF.Tanh, scale=0.5)
            nc.gpsimd.tensor_scalar(out=tho[:, 0:nb, :], in0=tho[:, 0:nb, :],
                                    scalar1=0.5, scalar2=0.5,
                                    op0=ALU.mult, op1=ALU.add)
            # blocks 16*c2.. -> pairs 8*c2.., t = parity
            og = gob8[:, 8 * c2:8 * c2 + nb // 2, :, 0:H]
            vi = v_ps[:, 0:nb, 0:H].rearrange("p (a b) h -> p a b h", b=2)
            ti = tho[:, 0:nb, :].rearrange("p (a b) h -> p a b h", b=2)
            nc.vector.tensor_tensor(out=og, in0=vi, in1=ti, op=ALU.mult)

        def agg_flush(g, agg, eng="v"):
            sl = slice(g * GW, (g + 1) * GW)
            if eng == "v":
                nc.vector.tensor_copy(ags[0:H + 1, sl], agg[0:H + 1, :])
            else:
                nc.scalar.copy(ags[0:H + 1, sl], agg[0:H + 1, :])

        # ---------------- epilogue ----------------
        def epi_half(half):
            """Batched epilogue for 8 map tiles: all PE matmuls first
            (into two grouped PSUM tiles, 512B-strided so every [128,66]
            output stays within one bank), then the elementwise chain
            pipelines across tiles on DVE/ACT without PE round-trips."""
            base = half * (NT // 2)
            uda = ps.tile([P, 8, P], F32, tag="st", name=f"uda{half}")
            gpa = ps.tile([P, 8, P], F32, tag="st", name=f"gpa{half}")
            for i in range(8):
                sl = slice((base + i) * P, (base + i + 1) * P)
                nc.tensor.matmul(uda[:, i, 0:E + 2], ags[:, sl], woe,
                                 start=True, stop=True)
                nc.tensor.matmul(gpa[:, i, 0:E + 2], gmT[:, sl], woe,
                                 start=True, stop=True)
            # evacuate PSUM immediately (uda/gpa sit in the ST rotation --
            # holding them through the elementwise chain would strangle the
            # main loop when this runs inside hp1)
            uds = sb_sm.tile([P, 8, E + 2], F32, tag="uds", name=f"uds{half}")
            nc.vector.tensor_copy(uds, uda[:, :, 0:E + 2])
            gxs = sb_sm.tile([P, 8, E + 2], F32, tag="gxs", name=f"gxs{half}")
            nc.scalar.copy(gxs, gpa[:, :, 0:E + 2])
            rden = sb_sm.tile([P, 8], F32, tag="rden", name=f"rden{half}")
            nc.vector.reciprocal(rden, uds[:, :, E])
            for i in range(8):
                t = base + i
                ut = sb_sm.tile([P, E], F32, tag="ut", name=f"ut{t}")
                nc.vector.scalar_tensor_tensor(out=ut, in0=gxs[:, i, 0:E],
                                               scalar=gxs[:, i, E:E + 1],
                                               in1=uds[:, i, 0:E],
                                               op0=ALU.mult, op1=ALU.add)
                nc.vector.scalar_tensor_tensor(out=out_pre[:, t, :], in0=ut,
                                               scalar=rden[:, i:i + 1],
                                               in1=map_pb[:, t, :],
                                               op0=ALU.mult, op1=ALU.add)
                stats = sb_sm.tile([P, 6], F32, tag="stats", name=f"stats{t}")
                nc.vector.bn_stats(stats, out_pre[:, t, :])
                nc.vector.bn_aggr(mvC[:, t, :], stats)

        def epi_final(half, act_assist=True):
            tsl = slice(half * (NT // 2), (half + 1) * (NT // 2))
            w = NT // 2
            vpe = sb_sm.tile([P, w], F32, tag="vpe", name=f"vpe{half}")
            nc.vector.tensor_scalar_add(vpe, mvC[:, tsl, 1], EPS)
            c1 = sb_sm.tile([P, w], F32, tag="nc1", name=f"nc1{half}")
            nc.vector.tensor_scalar(out=c1, in0=vpe, scalar1=0.564185,
                                    scalar2=0.378467, op0=ALU.mult,
                                    op1=ALU.add)
            c2 = sb_sm.tile([P, w], F32, tag="nc2", name=f"nc2{half}")
            nc.vector.tensor_scalar(out=c2, in0=vpe, scalar1=0.288949,
                                    scalar2=0.791321, op0=ALU.mult,
                                    op1=ALU.add)
            nc.vector.tensor_tensor(out=c1, in0=c1, in1=c2, op=ALU.min)
            rs = rstd[:, tsl]
            nc.vector.reciprocal(rs, c1)
            for _ in range(3):
                nc.vector.tensor_tensor(out=c1, in0=rs, in1=rs, op=ALU.mult)
                nc.vector.tensor_tensor(out=c1, in0=c1, in1=vpe, op=ALU.mult)
                nc.vector.tensor_scalar(out=c1, in0=c1, scalar1=-0.5,
                                        scalar2=1.5, op0=ALU.mult,
                                        op1=ALU.add)
                nc.vector.tensor_tensor(out=rs, in0=rs, in1=c1, op=ALU.mult)
            for t in range(half * (NT // 2), (half + 1) * (NT // 2)):
                xn = sb_sm.tile([P, E], F32, tag="xn", name=f"xn{t}")
                if act_assist:
                    nmr = sb_sm.tile([P, 1], F32, tag="nmr", name=f"nmr{t}")
                    nc.vector.tensor_scalar(out=nmr, in0=mvC[:, t, 0:1],
                                            scalar1=rstd[:, t:t + 1],
                                            scalar2=-1.0, op0=ALU.mult,
                                            op1=ALU.mult)
                    nc.scalar.activation(xn, out_pre[:, t, :], AF.Identity,
                                         bias=nmr, scale=rstd[:, t:t + 1])
                else:
                    nc.vector.tensor_scalar(out=xn, in0=out_pre[:, t, :],
                                            scalar1=mvC[:, t, 0:1],
                                            scalar2=rstd[:, t:t + 1],
                                            op0=ALU.subtract, op1=ALU.mult)
                nc.gpsimd.tensor_tensor(out=xn, in0=xn, in1=ga_b, op=ALU.mult)
                nc.gpsimd.tensor_tensor(out=out_all[:, t, :], in0=xn,
                                        in1=be_b, op=ALU.add)
            od = out_d.rearrange("(t p) e -> p t e", p=P)
            for q in range(2):
                qsl = slice(half * (NT // 2) + q * (NT // 4),
                            half * (NT // 2) + (q + 1) * (NT // 4))
                nc.sync.dma_start(od[:, qsl, :], out_all[:, qsl, :])

        # ---------------- prologue head ----------------
        map_chunk(0)
        map_chunk(1)
        obs_k_chunk2(0)
        obs_v_batch2(0)
        map_chunk(2)
        map_chunk(3)

        # drip the remaining prologue into the first half-pass
        # drip schedule. IMPORTANT: obs_v_batch stays a single drip unit --
        # its PSUM tile comes from the shared rotating "st" tag, so the glu
        # must read it before the main loop's next st allocations wrap
        # around the pool and clobber the bank.
        drip = {}
        items = []
        for c2 in range(1, NO // GW // 2):
            items.append((4 * (c2 - 1), lambda c2=c2: obs_k_chunk2(c2)))
        for b2 in range(1, 4):
            items.append((7 * b2 - 3, lambda b2=b2: obs_v_batch2(b2)))
        items.sort(key=lambda x: x[0])
        used = set()
        for want, fn in items:
            pp = want
            while pp in used:
                pp += 1
            used.add(pp)
            drip.setdefault(pp, []).append(fn)

        # exp unit assignment: alternate engines per (pair, group); bias
        # toward ACT by granting it both groups every few pairs.
        def exp_unit(st_t, pt_t, eng):
            if eng == "a":
                nc.scalar.activation(pt_t, st_t, AF.Exp,
                                     scale=1.0 / TEMP, bias=msh)
            else:
                nc.vector.tensor_scalar(out=pt_t.bitcast(U8), in0=st_t,
                                        scalar1=SCH_A, scalar2=SCH_B,
                                        op0=ALU.mult, op1=ALU.add)

        # ---------------- main loop: 2 half-passes x 32 pairs ----------
        # Software-pipelined by one pair: the PV for pair p-1 is issued to
        # the PE AFTER pair p's ST matmuls, so by the time the PE FIFO
        # reaches it, exp(p-1) has long finished -- no head-of-line stall.
        for hp in range(2):
            agg0 = ps_agg.tile([MPAD, GW], F32, tag="agg", name=f"agg{hp}_0")
            agg1 = ps_agg.tile([MPAD, GW], F32, tag="agg", name=f"agg{hp}_1")
            g0 = 2 * hp
            g1 = 2 * hp + 1
            s0 = slice(g0 * GW, (g0 + 1) * GW)
            s1 = slice(g1 * GW, (g1 + 1) * GW)
            nc.tensor.matmul(agg0, sel66, gmT[H:H + 1, s0],
                             start=True, stop=False)
            nc.tensor.matmul(agg1, sel66, gmT[H:H + 1, s1],
                             start=True, stop=False)
            prev_pt = None
            for pp in range(NPAIR):
                st0 = ps.tile([P, 2, GW], F32, tag="st", name=f"st{hp}_{pp}_0")
                st1 = ps.tile([P, 2, GW], F32, tag="st", name=f"st{hp}_{pp}_1")
                ko_lo = okT[0:E, pp, :]
                ko_hi = okT[E:P, pp, :]
                nc.tensor.matmul(st0[:, 0, :], ko_lo, qT[0:E, s0],
                                 start=True, stop=True)
                nc.tensor.matmul(st0[:, 1, :], ko_hi, qT[E:P, s0],
                                 start=True, stop=True)
                nc.tensor.matmul(st1[:, 0, :], ko_lo, qT[0:E, s1],
                                 start=True, stop=True)
                nc.tensor.matmul(st1[:, 1, :], ko_hi, qT[E:P, s1],
                                 start=True, stop=True)
                if prev_pt is not None:
                    qq, qt0, qt1 = prev_pt
                    go = gob8[:, qq, :, :]
                    nc.tensor.matmul(agg0, go, qt0, start=False, stop=False,
                                     perf_mode=DR)
                    nc.tensor.matmul(agg1, go, qt1, start=False, stop=False,
                                     perf_mode=DR)
                pt0 = sb_pt.tile([P, 2, GW], FP8E5, tag="pt",
                                 name=f"pt{hp}_{pp}_0")
                pt1 = sb_pt.tile([P, 2, GW], FP8E5, tag="pt",
                                 name=f"pt{hp}_{pp}_1")
                # exp split: in hp0 DVE also carries the drip (casts/glu),
                # so ACT takes both groups every 4th pair; in hp1 the
                # engines are evenly loaded, so strict 1:1.
                bonus = (pp % 4 == 3) if hp == 0 else False
                exp_unit(st0, pt0, "a")
                exp_unit(st1, pt1, "a" if bonus else "v")
                prev_pt = (pp, pt0, pt1)
                if hp == 0:
                    for fn in drip.get(pp, ()):
                        fn()
                else:
                    # hp0's ags columns are final: run its epilogue during
                    # hp1 (batched -- only two extra PSUM allocs total)
                    if pp == 6:
                        epi_half(0)
                    elif pp == 16:
                        epi_final(0)
            qq, qt0, qt1 = prev_pt
            go = gob8[:, qq, :, :]
            nc.tensor.matmul(agg0, go, qt0, start=False, stop=True,
                             perf_mode=DR)
            nc.tensor.matmul(agg1, go, qt1, start=False, stop=True,
                             perf_mode=DR)
            agg_flush(g0, agg0, eng="v")
            agg_flush(g1, agg1, eng="a")

        # ---------------- epilogue (half 0 ran during hp1) ----------
        epi_half(1)
        epi_final(1)

        if dbg is not None:
            nc.sync.dma_start(dbg["qT"], qT)
            nc.sync.dma_start(dbg["gmT"], gmT)
            nc.sync.dma_start(dbg["ags"], ags)
            nc.sync.dma_start(dbg["okT"], okT.rearrange("p a b -> p (a b)"))
            nc.sync.dma_start(dbg["gob8"],
                              gob8.rearrange("p a b c -> p (a b c)"))
            nc.sync.dma_start(dbg["out_pre"],
                              out_pre.rearrange("p a b -> p (a b)"))
            nc.sync.dma_start(dbg["mvC"], mvC.rearrange("p a b -> p (a b)"))


_CACHED = None


def _build(debug=False):
    global _CACHED
    if _CACHED is not None and not debug:
        return _CACHED
    nc = bacc.Bacc("TRN2", target_bir_lowering=False, debug=False)

    def din(name, shape, dt=F32):
        return nc.dram_tensor(name, shape, dt, kind="ExternalInput").ap()

    map_rows_d = din("map_rows", [NS, E])
    mapT_d = din("mapT", [E, NS], BF16)
    obsT_d = din("obsT", [E, NO], BF16)
    wpb_d = din("wpb", [E, BW], BF16)
    wpf_d = din("wpf", [E, FW], F32R)
    c8_d = din("c8", [P, MPAD - H], FP8)
    vec_d = din("vpack", [3 * E + 1])
    out_d = nc.dram_tensor("out", [NS, E], F32, kind="ExternalOutput").ap()

    dbg = None
    if debug:
        def dout(name, shape, dt=F32):
            return nc.dram_tensor(name, shape, dt, kind="ExternalOutput").ap()
        dbg = {
            "qT": dout("dbg_qT", [P, NS], F32R),
            "gmT": dout("dbg_gmT", [H + 1, NS], F32R),
            "ags": dout("dbg_ags", [H + 1, NS], F32R),
            "okT": dout("dbg_okT", [P, NPAIR * P], F32R),
            "gob8": dout("dbg_gob8", [P, NPAIR * 2 * MPAD], FP8),
            "out_pre": dout("dbg_out_pre", [P, NT * E]),
            "mvC": dout("dbg_mvC", [P, NT * 2]),
        }

    with tile.TileContext(nc) as tc:
        _emit(tc, out_d, map_rows_d, mapT_d, obsT_d, wpb_d, wpf_d, c8_d,
              vec_d, dbg=dbg)
    nc.compile()
    if not debug:
        _CACHED = nc
    return nc


def _prep_in_maps(map_code, obs_code, Wq, Wk, Wv, Wo, bo, gamma, beta):
    f = np.float32
    map_code = np.ascontiguousarray(np.asarray(map_code, dtype=f))
    obs_code = np.asarray(obs_code, dtype=f)

    bf16_np = mybir.dt.np(BF16)
    fp8_np = mybir.dt.np(FP8)

    def to_bf16(x):
        return np.ascontiguousarray(np.asarray(x, dtype=f).astype(bf16_np))

    obsT = np.ascontiguousarray(obs_code.T)

    wq2 = np.concatenate([np.asarray(Wq, f), np.asarray(Wq, f)], axis=1)
    wk2 = np.concatenate([np.asarray(Wk, f), np.asarray(Wk, f)], axis=1)
    woe = np.zeros((E, E + 2), dtype=f)
    woe[0:H, 0:E] = np.asarray(Wo, dtype=f)
    woe[H, E] = 1.0
    wpb = np.zeros((E, BW), dtype=f)
    wpb[:, _WQ0:_WQ0 + 128] = wq2
    wpb[:, _WK0:_WK0 + 128] = wk2
    wpb[:, _WV0:_WV0 + E] = np.asarray(Wv, f)

    wpf = np.zeros((E, FW), dtype=f)
    wpf[:, _ONES0] = 1.0
    wpf[H, _SEL0 + H] = 1.0   # sel row: 1.0 at position 32 (partition 32)
    wpf[:, _WOE0:_WOE0 + E + 2] = woe

    c8 = np.zeros((P, MPAD - H), dtype=fp8_np)
    c8[:, 0] = 1.0

    vpack = np.concatenate([
        np.asarray(bo, dtype=f), np.asarray(gamma, dtype=f),
        np.asarray(beta, dtype=f), np.full((1,), -SHIFT, dtype=f),
    ])
    shared = {
        "obsT": to_bf16(obsT),
        "wpb": to_bf16(wpb),
        "wpf": np.ascontiguousarray(wpf),
        "c8": np.ascontiguousarray(c8),
        "vpack": np.ascontiguousarray(vpack),
    }
    in_maps = []
    for i in range(NCORES):
        shard = map_code[i * NS:(i + 1) * NS]
        m = dict(shared)
        m["map_rows"] = shard
        m["mapT"] = to_bf16(np.ascontiguousarray(shard.T))
        in_maps.append(m)
    return in_maps


def run(trace=False, **inputs):
    nc = _build()
    in_maps = _prep_in_maps(**inputs)
    res = run_bass_kernel_spmd(nc, in_maps, list(range(NCORES)), trace=trace)
    out = np.concatenate([res.results[i]["out"] for i in range(NCORES)], axis=0)
    return out, res


def kernel(**inputs):
    out, _ = run(trace=False, **inputs)
    return out


# revision 29
# speedup vs baseline: 1.0859x; 1.0859x over previous
"""Trainium2 Bass kernel for nn_Attention_5815385719367 (gnn_message_passing).

Computation (see reference):
  map_q/k/v = map_code @ Wq/Wk/Wv ; obs_k/v = obs_code @ Wk/Wv
  scores    = [sum(q*k,-1) | q @ obs_k.T] / 8
  w         = softmax(scores)
  agg       = w[:, :1]*glu(map_v) + w[:, 1:] @ glu(obs_v)
  out       = LN(agg @ Wo + bo + map_code) * gamma + beta

Sharding: data-parallel over N_map rows (2048 rows/core x 8 cores);
obs_code and weights replicated. No collectives.

v2 design notes (per core):
  - scores computed TRANSPOSED in PSUM: ST[obs=128, map] via PE ROW-TILED
    pairs: even obs block's k.T sits on SBUF partitions 0-63 (PE tile
    (0,0)), odd block's on partitions 64-127 (tile (64,0)); the two
    256-cycle streams run CONCURRENTLY in different PE row groups, so
    ST costs ~0.5 cyc/col. qT and okT are produced pre-duplicated /
    parity-split by projecting with host-duplicated weights
    ([64,128] wq|wq and wk|wk), so no cross-partition copies exist.
  - softmax exp is the hard wall (1 elem/lane/cycle on ACT): split it
    between ACT (direct exp -> fp8e4m3, logits shifted by -1 so
    exp <= ~110 < 240 = trn-e4m3 max) and DVE (Schraudolph: one
    mult-add tensor_scalar into uint8, whose bit pattern IS the
    e4m3 log-domain approximation; errors ~+-5% are noise-like and
    average out over 8k obs).
  - PV runs fp8 DoubleRow over block PAIRS: stationary
    gob8[128, 2, 66] = glu(obs_v)|ones|zero-pad for (even, odd)
    blocks, moving pt8[128, 2, 512] = exp'd scores; contraction is
    256 at 0.5 cyc/col. M padded 33->66 keeps col tiling off
    (DoubleRow is incompatible with column tiling). The ones column
    accumulates the softmax denominator for free.
  - self-attention term folded in after Wo (as v1): agg row 32 is
    seeded with selfexp, epilogue adds selfexp*(glu(map_v)@Wo) and
    divides by the denominator; the -1 logit shift cancels exactly.
  - projections in bf16 (inputs shipped bf16), epilogue Wo in bf16.
  - GPSIMD does SBUF-side elementwise work (sigmoid affine steps,
    map+bo, gamma/beta) since it cannot touch PSUM.
"""

import numpy as np

import concourse.bass as bass
import concourse.bacc as bacc
import concourse.tile as tile
from concourse import mybir
from concourse.bass_utils import run_bass_kernel_spmd

NCORES = 8
NM, NO, E = 16384, 8192, 64
NS = NM // NCORES            # 2048 map rows per core
H = E // 2                   # 32
TEMP = 8.0
EPS = 1e-6
P = 128
NT = NS // P                 # 16 row tiles per core
GW = 512                     # map group width (psum bank)
NPAIR = NO // 256            # 32 obs block-pairs
SHIFT = -2.0                 # logit shift: exp(l - SHIFT), cancels in ratio.
                             # Logits empirically span [-8.1, 8.32]; e5m2's
                             # 22-e-fold range with shift -2 covers all of it
                             # (max exp ~ e^10.3 = 3e4 < 57344) with no
                             # clipping at either end.
MPAD = 80                    # padded PV output partitions (33 real; %16 keeps the DoubleRow ldweights step legal, >64 keeps column tiling off)

F32 = mybir.dt.float32
F32R = mybir.dt.float32r
BF16 = mybir.dt.bfloat16
FP8 = mybir.dt.float8e4
FP8E5 = mybir.dt.float8e5
U8 = mybir.dt.uint8
AF = mybir.ActivationFunctionType
ALU = mybir.AluOpType
DR = mybir.MatmulPerfMode.DoubleRow

# Schraudolph constants for uint8 e5m2 log-domain exp of RAW score s:
#   i = 4*log2(exp(s/8 - SHIFT)) + 60 - sawtooth_center
# DVE float->uint8 conversion saturates [0,255] and rounds RNE (probed).
SCH_A = 4.0 * 1.4426950408889634 / TEMP   # 0.72135
SCH_B = 60.0 - 4.0 * 1.4426950408889634 * SHIFT - 0.229

# layout of the bf16 weight pack [64, BW]
_WQ0 = 0              # wq duplicated [64, 128]
_WK0 = 128            # wk duplicated [64, 128]
_WV0 = 256            # wv [64, 64]
BW = 320

# layout of the f32r pack [64, FW]
_ONES0 = 0            # ones column [64, 1]
_SEL0 = 1             # sel row at partition 32: [1, 66]
_WOE0 = 1 + MPAD      # woe [33 rows used, 66]
FW = 1 + MPAD + E + 2


def _bc_part(ap, n):
    """Broadcast a [x, ...] AP along a new leading partition dim of n."""
    return bass.AP(tensor=ap.tensor, offset=ap.offset, ap=[[0, n]] + list(ap.ap))


def _emit(tc, out_d, map_rows_d, mapT_d, obsT_d, wpb_d, wpf_d, c8_d, vec_d,
          dbg=None, exp_act_frac=0.5):
    nc = tc.nc
    with tc.tile_pool(name="consts", bufs=1) as consts, \
         tc.tile_pool(name="big", bufs=1) as big, \
         tc.tile_pool(name="sb_sm", bufs=3) as sb_sm, \
         tc.tile_pool(name="sb_pt", bufs=4) as sb_pt, \
         tc.tile_pool(name="ps", bufs=3, space="PSUM") as ps, \
         tc.tile_pool(name="ps_agg", bufs=2, space="PSUM") as ps_agg:

        # ---------------- constants ----------------
        wpb = consts.tile([E, BW], BF16)          # bf16 weights pack
        nc.sync.dma_start(wpb, wpb_d)
        wq2 = wpb[:, _WQ0:_WQ0 + 128]             # [64,128] wq|wq
        wk2 = wpb[:, _WK0:_WK0 + 128]             # [64,128] wk|wk
        wv = wpb[:, _WV0:_WV0 + E]                # [64,64]

        wpf = consts.tile([E, FW], F32R)
        nc.sync.dma_start(wpf, wpf_d)
        ones64 = wpf[:, _ONES0:_ONES0 + 1]
        sel66 = wpf[H:H + 1, _SEL0:_SEL0 + MPAD]  # row at partition 32
        woe = wpf[0:H + 1, _WOE0:_WOE0 + E + 2]   # [33,66]

        vecs = consts.tile([P, 3 * E + 1], F32)   # bo|gamma|beta|-shift
        nc.sync.dma_start(vecs, _bc_part(vec_d, P))
        bo_b = vecs[:, 0:E]
        ga_b = vecs[:, E:2 * E]
        be_b = vecs[:, 2 * E:3 * E]
        msh = vecs[:, 3 * E:3 * E + 1]            # -SHIFT bias column

        # ---------------- big arenas + input DMAs ----------------
        mapT = big.tile([E, NS], BF16)
        obsT = big.tile([E, NO], BF16)
        for lo, hi, t_, s_ in ((0, 512, mapT, mapT_d),
                               (0, 1024, obsT, obsT_d),
                               (512, 1024, mapT, mapT_d),
                               (1024, 2048, obsT, obsT_d),
                               (1024, 2048, mapT, mapT_d),
                               (2048, 4096, obsT, obsT_d),
                               (4096, 8192, obsT, obsT_d)):
            nc.sync.dma_start(t_[:, lo:hi], s_[:, lo:hi])
        map_rows = big.tile([P, NT, E], F32)
        nc.sync.dma_start(map_rows, map_rows_d.rearrange("(t p) e -> p t e", p=P))

        qT = big.tile([P, NS], F32R)              # map_q.T duplicated halves
        gmT = big.tile([H + 1, NS], F32R)         # [glu(map_v).T ; selfexp]
        okT = big.tile([P, NPAIR, P], F32R)       # obs_k.T parity-split
        gob8 = big.tile([P, NPAIR, 2, MPAD], FP8)  # glu(obs_v)|1|0 pairs
        ags = big.tile([H + 1, NS], F32R)         # [numer.T ; denom]
        map_pb = big.tile([P, NT, E], F32)        # map + bo
        out_pre = big.tile([P, NT, E], F32)
        out_all = big.tile([P, NT, E], F32)
        mvC = big.tile([P, NT, 2], F32)
        rstd = big.tile([P, NT], F32)

        # gob8 static columns: ones at h=32, zeros at h=33..65 (DMA from
        # the small HBM consts tensor, replicated via zero strides)
        gob8f = gob8.rearrange("p a b c -> p (a b) c")
        C8W = MPAD - H
        ones_src = bass.AP(tensor=c8_d.tensor, offset=c8_d.offset,
                           ap=[[C8W, P], [0, 2 * NPAIR], [0, 1]])
        nc.sync.dma_start(gob8f[:, :, H:H + 1], ones_src)
        zero_src = bass.AP(tensor=c8_d.tensor, offset=c8_d.offset + 1,
                           ap=[[C8W, P], [0, 2 * NPAIR], [1, MPAD - H - 1]])
        nc.sync.dma_start(gob8f[:, :, H + 1:MPAD], zero_src)

        # map + bo on gpsimd (all-SBUF)
        bo_rep = bass.AP(tensor=bo_b.tensor, offset=bo_b.offset,
                         ap=[list(bo_b.ap[0]), [0, NT], [1, E]])
        nc.gpsimd.tensor_tensor(out=map_pb, in0=map_rows, in1=bo_rep,
                                op=ALU.add)

        # ---------------- prologue pieces ----------------
        def map_chunk(c):
            """q (duplicated), selfexp, glu(map_v) for map cols [c*512, ..)."""
            sl = slice(c * GW, (c + 1) * GW)
            q_ps = ps.tile([P, 2, GW], F32, tag="st", name=f"qps{c}")
            nc.tensor.matmul(q_ps[:, 0, :], wq2, mapT[:, sl],
                             start=True, stop=True)
            nc.vector.tensor_copy(qT[:, sl], q_ps[:, 0, :])
            k_ps = ps.tile([P, 2, GW], F32, tag="st", name=f"kps{c}")
            nc.tensor.matmul(k_ps[:, 0, :], wk2, mapT[:, sl],
                             start=True, stop=True)
            qk = sb_sm.tile([E, GW], F32R, tag="qk", name=f"qk{c}")
            nc.vector.tensor_tensor(out=qk, in0=qT[0:E, sl],
                                    in1=k_ps[0:E, 0, :], op=ALU.mult)
            # self-score sum lands in the unused upper half of k_ps
            ss_ps = k_ps[0:1, 1, :]
            nc.tensor.matmul(ss_ps, ones64, qk, start=True, stop=True)
            nc.scalar.activation(gmT[H:H + 1, sl], ss_ps, AF.Exp,
                                 scale=1.0 / TEMP, bias=msh[0:1])
            v_ps = ps.tile([P, 2, GW], F32, tag="st", name=f"vps{c}")
            nc.tensor.matmul(v_ps[0:E, 0, :], wv, mapT[:, sl],
                             start=True, stop=True)
            th = sb_sm.tile([H, GW], F32, tag="th", name=f"th{c}")
            nc.scalar.activation(th, v_ps[H:E, 0, :], AF.Tanh, scale=0.5)
            nc.gpsimd.tensor_scalar(out=th, in0=th, scalar1=0.5, scalar2=0.5,
                                    op0=ALU.mult, op1=ALU.add)
            nc.vector.tensor_tensor(out=gmT[0:H, sl], in0=v_ps[0:H, 0, :],
                                    in1=th, op=ALU.mult)

        def obs_k_chunk2(c2, eng="v"):
            """okT parity-split fill for TWO obs chunks (one PSUM alloc).

            chunk c covers obs cols [c*512, ..) = blocks 4c..4c+3; even
            blocks land on partitions 0-63 of okT, odd blocks on 64-127
            (via the duplicated upper half of the wk2 projection, so no
            cross-partition movement is needed)."""
            k_ps = ps.tile([P, 2, GW], F32, tag="st", name=f"okps{c2}")
            for t in range(2):
                c = 2 * c2 + t
                sl = slice(c * GW, (c + 1) * GW)
                nc.tensor.matmul(k_ps[:, t, :], wk2, obsT[:, sl],
                                 start=True, stop=True)
            for t in range(2):
                c = 2 * c2 + t
                ev_in = k_ps[0:E, t, :].rearrange("p (b m) -> p b m",
                                                  b=2)[:, :, 0:P]
                ev_out = okT[0:E, 2 * c:2 * c + 2, :]
                od_in = k_ps[E:P, t, :].rearrange("p (b m) -> p b m",
                                                  b=2)[:, :, P:2 * P]
                od_out = okT[E:P, 2 * c:2 * c + 2, :]
                if eng == "v":
                    nc.vector.tensor_copy(ev_out, ev_in)
                    nc.vector.tensor_copy(od_out, od_in)
                else:
                    nc.scalar.copy(ev_out, ev_in)
                    nc.scalar.copy(od_out, od_in)

        def obs_v_batch2(c2, nb=16):
            """glu(obs_v) for nb consecutive obs blocks (one PSUM alloc)."""
            v_ps = ps.tile([P, 16, E], F32, tag="st", name=f"ovps{c2}")
            for b in range(nb):
                blk = c2 * 16 + b
                nc.tensor.matmul(v_ps[:, b, :],
                                 obsT[:, blk * P:(blk + 1) * P], wv,
                                 start=True, stop=True)
            tho = sb_sm.tile([P, 16, H], F32, tag="tho", name=f"tho{c2}")
            nc.scalar.activation(tho[:, 0:nb, :], v_ps[:, 0:nb, H:E],
                                 AF.Tanh, scale=0.5)
            nc.gpsimd.tensor_scalar(out=tho[:, 0:nb, :], in0=tho[:, 0:nb, :],
                                    scalar1=0.5, scalar2=0.5,
                                    op0=ALU.mult, op1=ALU.add)
            # blocks 16*c2.. -> pairs 8*c2.., t = parity
            og = gob8[:, 8 * c2:8 * c2 + nb // 2, :, 0:H]
            vi = v_ps[:, 0:nb, 0:H].rearrange("p (a b) h -> p a b h", b=2)
            ti = tho[:, 0:nb, :].rearrange("p (a b) h -> p a b h", b=2)
            nc.vector.tensor_tensor(out=og, in0=vi, in1=ti, op=ALU.mult)

        def agg_flush(g, agg, eng="v"):
            sl = slice(g * GW, (g + 1) * GW)
            if eng == "v":
                nc.vector.tensor_copy(ags[0:H + 1, sl], agg[0:H + 1, :])
            else:
                nc.scalar.copy(ags[0:H + 1, sl], agg[0:H + 1, :])

        # ---------------- epilogue ----------------
        def epi_half(half):
            """Batched epilogue for 8 map tiles: all PE matmuls first
            (into two grouped PSUM tiles, 512B-strided so every [128,66]
            output stays within one bank), then the elementwise chain
            pipelines across tiles on DVE/ACT without PE round-trips."""
            base = half * (NT // 2)
            uda = ps.tile([P, 8, P], F32, tag="st", name=f"uda{half}")
            gpa = ps.tile([P, 8, P], F32, tag="st", name=f"gpa{half}")
            for i in range(8):
                sl = slice((base + i) * P, (base + i + 1) * P)
                nc.tensor.matmul(uda[:, i, 0:E + 2], ags[:, sl], woe,
                                 start=True, stop=True)
                nc.tensor.matmul(gpa[:, i, 0:E + 2], gmT[:, sl], woe,
                                 start=True, stop=True)
            # evacuate PSUM immediately (uda/gpa sit in the ST rotation --
            # holding them through the elementwise chain would strangle the
            # main loop when this runs inside hp1)
            uds = sb_sm.tile([P, 8, E + 2], F32, tag="uds", name=f"uds{half}")
            nc.scalar.copy(uds, uda[:, :, 0:E + 2])
            gxs = sb_sm.tile([P, 8, E + 2], F32, tag="gxs", name=f"gxs{half}")
            nc.scalar.copy(gxs, gpa[:, :, 0:E + 2])
            rden = sb_sm.tile([P, 8], F32, tag="rden", name=f"rden{half}")
            nc.vector.reciprocal(rden, uds[:, :, E])
            for i in range(8):
                t = base + i
                ut = sb_sm.tile([P, E], F32, tag="ut", name=f"ut{t}")
                nc.vector.scalar_tensor_tensor(out=ut, in0=gxs[:, i, 0:E],
                                               scalar=gxs[:, i, E:E + 1],
                                               in1=uds[:, i, 0:E],
                                               op0=ALU.mult, op1=ALU.add)
                nc.vector.scalar_tensor_tensor(out=out_pre[:, t, :], in0=ut,
                                               scalar=rden[:, i:i + 1],
                                               in1=map_pb[:, t, :],
                                               op0=ALU.mult, op1=ALU.add)
                stats = sb_sm.tile([P, 6], F32, tag="stats", name=f"stats{t}")
                nc.vector.bn_stats(stats, out_pre[:, t, :])
                nc.vector.bn_aggr(mvC[:, t, :], stats)

        def epi_final(half, act_assist=True):
            tsl = slice(half * (NT // 2), (half + 1) * (NT // 2))
            w = NT // 2
            vpe = sb_sm.tile([P, w], F32, tag="vpe", name=f"vpe{half}")
            nc.vector.tensor_scalar_add(vpe, mvC[:, tsl, 1], EPS)
            c1 = sb_sm.tile([P, w], F32, tag="nc1", name=f"nc1{half}")
            nc.vector.tensor_scalar(out=c1, in0=vpe, scalar1=0.564185,
                                    scalar2=0.378467, op0=ALU.mult,
                                    op1=ALU.add)
            c2 = sb_sm.tile([P, w], F32, tag="nc2", name=f"nc2{half}")
            nc.vector.tensor_scalar(out=c2, in0=vpe, scalar1=0.288949,
                                    scalar2=0.791321, op0=ALU.mult,
                                    op1=ALU.add)
            nc.vector.tensor_tensor(out=c1, in0=c1, in1=c2, op=ALU.min)
            rs = rstd[:, tsl]
            nc.vector.reciprocal(rs, c1)
            for _ in range(3):
                nc.vector.tensor_tensor(out=c1, in0=rs, in1=rs, op=ALU.mult)
                nc.vector.tensor_tensor(out=c1, in0=c1, in1=vpe, op=ALU.mult)
                nc.vector.tensor_scalar(out=c1, in0=c1, scalar1=-0.5,
                                        scalar2=1.5, op0=ALU.mult,
                                        op1=ALU.add)
                nc.vector.tensor_tensor(out=rs, in0=rs, in1=c1, op=ALU.mult)
            for t in range(half * (NT // 2), (half + 1) * (NT // 2)):
                xn = sb_sm.tile([P, E], F32, tag="xn", name=f"xn{t}")
                if act_assist:
                    nmr = sb_sm.tile([P, 1], F32, tag="nmr", name=f"nmr{t}")
                    nc.vector.tensor_scalar(out=nmr, in0=mvC[:, t, 0:1],
                                            scalar1=rstd[:, t:t + 1],
                                            scalar2=-1.0, op0=ALU.mult,
                                            op1=ALU.mult)
                    nc.scalar.activation(xn, out_pre[:, t, :], AF.Identity,
                                         bias=nmr, scale=rstd[:, t:t + 1])
                else:
                    nc.vector.tensor_scalar(out=xn, in0=out_pre[:, t, :],
                                            scalar1=mvC[:, t, 0:1],
                                            scalar2=rstd[:, t:t + 1],
                                            op0=ALU.subtract, op1=ALU.mult)
                nc.gpsimd.tensor_tensor(out=xn, in0=xn, in1=ga_b, op=ALU.mult)
                nc.gpsimd.tensor_tensor(out=out_all[:, t, :], in0=xn,
                                        in1=be_b, op=ALU.add)
            od = out_d.rearrange("(t p) e -> p t e", p=P)
            for q in range(2):
                qsl = slice(half * (NT // 2) + q * (NT // 4),
                            half * (NT // 2) + (q + 1) * (NT // 4))
                nc.sync.dma_start(od[:, qsl, :], out_all[:, qsl, :])

        # ---------------- prologue head ----------------
        map_chunk(0)
        map_chunk(1)
        obs_k_chunk2(0)
        obs_v_batch2(0)
        map_chunk(2)
        map_chunk(3)

        # drip the remaining prologue into the first half-pass
        # drip schedule. IMPORTANT: obs_v_batch stays a single drip unit --
        # its PSUM tile comes from the shared rotating "st" tag, so the glu
        # must read it before the main loop's next st allocations wrap
        # around the pool and clobber the bank.
        drip = {}
        items = []
        for c2 in range(1, NO // GW // 2):
            items.append((4 * (c2 - 1),
                          lambda c2=c2: obs_k_chunk2(c2, eng="av"[c2 % 2])))
        for b2 in range(1, 4):
            items.append((7 * b2 - 3, lambda b2=b2: obs_v_batch2(b2)))
        items.sort(key=lambda x: x[0])
        used = set()
        for want, fn in items:
            pp = want
            while pp in used:
                pp += 1
            used.add(pp)
            drip.setdefault(pp, []).append(fn)

        # exp unit assignment: alternate engines per (pair, group); bias
        # toward ACT by granting it both groups every few pairs.
        def exp_unit(st_t, pt_t, eng):
            if eng == "a":
                nc.scalar.activation(pt_t, st_t, AF.Exp,
                                     scale=1.0 / TEMP, bias=msh)
            else:
                nc.vector.tensor_scalar(out=pt_t.bitcast(U8), in0=st_t,
                                        scalar1=SCH_A, scalar2=SCH_B,
                                        op0=ALU.mult, op1=ALU.add)

        # ---------------- main loop: 2 half-passes x 32 pairs ----------
        # Software-pipelined by one pair: the PV for pair p-1 is issued to
        # the PE AFTER pair p's ST matmuls, so by the time the PE FIFO
        # reaches it, exp(p-1) has long finished -- no head-of-line stall.
        for hp in range(2):
            agg0 = ps_agg.tile([MPAD, GW], F32, tag="agg", name=f"agg{hp}_0")
            agg1 = ps_agg.tile([MPAD, GW], F32, tag="agg", name=f"agg{hp}_1")
            g0 = 2 * hp
            g1 = 2 * hp + 1
            s0 = slice(g0 * GW, (g0 + 1) * GW)
            s1 = slice(g1 * GW, (g1 + 1) * GW)
            nc.tensor.matmul(agg0, sel66, gmT[H:H + 1, s0],
                             start=True, stop=False)
            nc.tensor.matmul(agg1, sel66, gmT[H:H + 1, s1],
                             start=True, stop=False)
            prev_pt = None
            for pp in range(NPAIR):
                st0 = ps.tile([P, 2, GW], F32, tag="st", name=f"st{hp}_{pp}_0")
                st1 = ps.tile([P, 2, GW], F32, tag="st", name=f"st{hp}_{pp}_1")
                ko_lo = okT[0:E, pp, :]
                ko_hi = okT[E:P, pp, :]
                nc.tensor.matmul(st0[:, 0, :], ko_lo, qT[0:E, s0],
                                 start=True, stop=True)
                nc.tensor.matmul(st0[:, 1, :], ko_hi, qT[E:P, s0],
                                 start=True, stop=True)
                nc.tensor.matmul(st1[:, 0, :], ko_lo, qT[0:E, s1],
                                 start=True, stop=True)
                nc.tensor.matmul(st1[:, 1, :], ko_hi, qT[E:P, s1],
                                 start=True, stop=True)
                if prev_pt is not None:
                    qq, qt0, qt1 = prev_pt
                    go = gob8[:, qq, :, :]
                    nc.tensor.matmul(agg0, go, qt0, start=False, stop=False,
                                     perf_mode=DR)
                    nc.tensor.matmul(agg1, go, qt1, start=False, stop=False,
                                     perf_mode=DR)
                pt0 = sb_pt.tile([P, 2, GW], FP8E5, tag="pt",
                                 name=f"pt{hp}_{pp}_0")
                pt1 = sb_pt.tile([P, 2, GW], FP8E5, tag="pt",
                                 name=f"pt{hp}_{pp}_1")
                # exp split: in hp0 DVE also carries the drip (casts/glu),
                # so ACT takes both groups every 4th pair; in hp1 the
                # engines are evenly loaded, so strict 1:1.
                bonus = (pp % 4 == 3) if hp == 0 else False
                exp_unit(st0, pt0, "a")
                exp_unit(st1, pt1, "a" if bonus else "v")
                prev_pt = (pp, pt0, pt1)
                if hp == 0:
                    for fn in drip.get(pp, ()):
                        fn()
                else:
                    # hp0's ags columns are final: run its epilogue during
                    # hp1 (batched -- only two extra PSUM allocs total)
                    if pp == 6:
                        epi_half(0)
                    elif pp == 16:
                        epi_final(0)
            qq, qt0, qt1 = prev_pt
            go = gob8[:, qq, :, :]
            nc.tensor.matmul(agg0, go, qt0, start=False, stop=True,
                             perf_mode=DR)
            nc.tensor.matmul(agg1, go, qt1, start=False, stop=True,
                             perf_mode=DR)
            agg_flush(g0, agg0, eng="v")
            agg_flush(g1, agg1, eng="a")

        # ---------------- epilogue (half 0 ran during hp1) ----------
        epi_half(1)
        epi_final(1)

        if dbg is not None:
            nc.sync.dma_start(dbg["qT"], qT)
            nc.sync.dma_start(dbg["gmT"], gmT)
            nc.sync.dma_start(dbg["ags"], ags)
            nc.sync.dma_start(dbg["okT"], okT.rearrange("p a b -> p (a b)"))
            nc.sync.dma_start(dbg["gob8"],
                              gob8.rearrange("p a b c -> p (a b c)"))
            nc.sync.dma_start(dbg["out_pre"],
                              out_pre.rearrange("p a b -> p (a b)"))
            nc.sync.dma_start(dbg["mvC"], mvC.rearrange("p a b -> p (a b)"))


_CACHED = None


def _build(debug=False):
    global _CACHED
    if _CACHED is not None and not debug:
        return _CACHED
    nc = bacc.Bacc("TRN2", target_bir_lowering=False, debug=False)

    def din(name, shape, dt=F32):
        return nc.dram_tensor(name, shape, dt, kind="ExternalInput").ap()

    map_rows_d = din("map_rows", [NS, E])
    mapT_d = din("mapT", [E, NS], BF16)
    obsT_d = din("obsT", [E, NO], BF16)
    wpb_d = din("wpb", [E, BW], BF16)
    wpf_d = din("wpf", [E, FW], F32R)
    c8_d = din("c8", [P, MPAD - H], FP8)
    vec_d = din("vpack", [3 * E + 1])
    out_d = nc.dram_tensor("out", [NS, E], F32, kind="ExternalOutput").ap()

    dbg = None
    if debug:
        def dout(name, shape, dt=F32):
            return nc.dram_tensor(name, shape, dt, kind="ExternalOutput").ap()
        dbg = {
            "qT": dout("dbg_qT", [P, NS], F32R),
            "gmT": dout("dbg_gmT", [H + 1, NS], F32R),
            "ags": dout("dbg_ags", [H + 1, NS], F32R),
            "okT": dout("dbg_okT", [P, NPAIR * P], F32R),
            "gob8": dout("dbg_gob8", [P, NPAIR * 2 * MPAD], FP8),
            "out_pre": dout("dbg_out_pre", [P, NT * E]),
            "mvC": dout("dbg_mvC", [P, NT * 2]),
        }

    with tile.TileContext(nc) as tc:
        _emit(tc, out_d, map_rows_d, mapT_d, obsT_d, wpb_d, wpf_d, c8_d,
              vec_d, dbg=dbg)
    nc.compile()
    if not debug:
        _CACHED = nc
    return nc


def _prep_in_maps(map_code, obs_code, Wq, Wk, Wv, Wo, bo, gamma, beta):
    f = np.float32
    map_code = np.ascontiguousarray(np.asarray(map_code, dtype=f))
    obs_code = np.asarray(obs_code, dtype=f)

    bf16_np = mybir.dt.np(BF16)
    fp8_np = mybir.dt.np(FP8)

    def to_bf16(x):
        return np.ascontiguousarray(np.asarray(x, dtype=f).astype(bf16_np))

    obsT = np.ascontiguousarray(obs_code.T)

    wq2 = np.concatenate([np.asarray(Wq, f), np.asarray(Wq, f)], axis=1)
    wk2 = np.concatenate([np.asarray(Wk, f), np.asarray(Wk, f)], axis=1)
    woe = np.zeros((E, E + 2), dtype=f)
    woe[0:H, 0:E] = np.asarray(Wo, dtype=f)
    woe[H, E] = 1.0
    wpb = np.zeros((E, BW), dtype=f)
    wpb[:, _WQ0:_WQ0 + 128] = wq2
    wpb[:, _WK0:_WK0 + 128] = wk2
    wpb[:, _WV0:_WV0 + E] = np.asarray(Wv, f)

    wpf = np.zeros((E, FW), dtype=f)
    wpf[:, _ONES0] = 1.0
    wpf[H, _SEL0 + H] = 1.0   # sel row: 1.0 at position 32 (partition 32)
    wpf[:, _WOE0:_WOE0 + E + 2] = woe

    c8 = np.zeros((P, MPAD - H), dtype=fp8_np)
    c8[:, 0] = 1.0

    vpack = np.concatenate([
        np.asarray(bo, dtype=f), np.asarray(gamma, dtype=f),
        np.asarray(beta, dtype=f), np.full((1,), -SHIFT, dtype=f),
    ])
    shared = {
        "obsT": to_bf16(obsT),
        "wpb": to_bf16(wpb),
        "wpf": np.ascontiguousarray(wpf),
        "c8": np.ascontiguousarray(c8),
        "vpack": np.ascontiguousarray(vpack),
    }
    in_maps = []
    for i in range(NCORES):
        shard = map_code[i * NS:(i + 1) * NS]
        m = dict(shared)
        m["map_rows"] = shard
        m["mapT"] = to_bf16(np.ascontiguousarray(shard.T))
        in_maps.append(m)
    return in_maps


def run(trace=False, **inputs):
    nc = _build()
    in_maps = _prep_in_maps(**inputs)
    res = run_bass_kernel_spmd(nc, in_maps, list(range(NCORES)), trace=trace)
    out = np.concatenate([res.results[i]["out"] for i in range(NCORES)], axis=0)
    return out, res


def kernel(**inputs):
    out, _ = run(trace=False, **inputs)
    return out


# revision 30
# speedup vs baseline: 1.0863x; 1.0003x over previous
"""Trainium2 Bass kernel for nn_Attention_5815385719367 (gnn_message_passing).

Computation (see reference):
  map_q/k/v = map_code @ Wq/Wk/Wv ; obs_k/v = obs_code @ Wk/Wv
  scores    = [sum(q*k,-1) | q @ obs_k.T] / 8
  w         = softmax(scores)
  agg       = w[:, :1]*glu(map_v) + w[:, 1:] @ glu(obs_v)
  out       = LN(agg @ Wo + bo + map_code) * gamma + beta

Sharding: data-parallel over N_map rows (2048 rows/core x 8 cores);
obs_code and weights replicated. No collectives.

v2 design notes (per core):
  - scores computed TRANSPOSED in PSUM: ST[obs=128, map] via PE ROW-TILED
    pairs: even obs block's k.T sits on SBUF partitions 0-63 (PE tile
    (0,0)), odd block's on partitions 64-127 (tile (64,0)); the two
    256-cycle streams run CONCURRENTLY in different PE row groups, so
    ST costs ~0.5 cyc/col. qT and okT are produced pre-duplicated /
    parity-split by projecting with host-duplicated weights
    ([64,128] wq|wq and wk|wk), so no cross-partition copies exist.
  - softmax exp is the hard wall (1 elem/lane/cycle on ACT): split it
    between ACT (direct exp -> fp8e4m3, logits shifted by -1 so
    exp <= ~110 < 240 = trn-e4m3 max) and DVE (Schraudolph: one
    mult-add tensor_scalar into uint8, whose bit pattern IS the
    e4m3 log-domain approximation; errors ~+-5% are noise-like and
    average out over 8k obs).
  - PV runs fp8 DoubleRow over block PAIRS: stationary
    gob8[128, 2, 66] = glu(obs_v)|ones|zero-pad for (even, odd)
    blocks, moving pt8[128, 2, 512] = exp'd scores; contraction is
    256 at 0.5 cyc/col. M padded 33->66 keeps col tiling off
    (DoubleRow is incompatible with column tiling). The ones column
    accumulates the softmax denominator for free.
  - self-attention term folded in after Wo (as v1): agg row 32 is
    seeded with selfexp, epilogue adds selfexp*(glu(map_v)@Wo) and
    divides by the denominator; the -1 logit shift cancels exactly.
  - projections in bf16 (inputs shipped bf16), epilogue Wo in bf16.
  - GPSIMD does SBUF-side elementwise work (sigmoid affine steps,
    map+bo, gamma/beta) since it cannot touch PSUM.
"""

import numpy as np

import concourse.bass as bass
import concourse.bacc as bacc
import concourse.tile as tile
from concourse import mybir
from concourse.bass_utils import run_bass_kernel_spmd

NCORES = 8
NM, NO, E = 16384, 8192, 64
NS = NM // NCORES            # 2048 map rows per core
H = E // 2                   # 32
TEMP = 8.0
EPS = 1e-6
P = 128
NT = NS // P                 # 16 row tiles per core
GW = 512                     # map group width (psum bank)
NPAIR = NO // 256            # 32 obs block-pairs
SHIFT = -2.0                 # logit shift: exp(l - SHIFT), cancels in ratio.
                             # Logits empirically span [-8.1, 8.32]; e5m2's
                             # 22-e-fold range with shift -2 covers all of it
                             # (max exp ~ e^10.3 = 3e4 < 57344) with no
                             # clipping at either end.
MPAD = 80                    # padded PV output partitions (33 real; %16 keeps the DoubleRow ldweights step legal, >64 keeps column tiling off)

F32 = mybir.dt.float32
F32R = mybir.dt.float32r
BF16 = mybir.dt.bfloat16
FP8 = mybir.dt.float8e4
FP8E5 = mybir.dt.float8e5
U8 = mybir.dt.uint8
AF = mybir.ActivationFunctionType
ALU = mybir.AluOpType
DR = mybir.MatmulPerfMode.DoubleRow

# Schraudolph constants for uint8 e5m2 log-domain exp of RAW score s:
#   i = 4*log2(exp(s/8 - SHIFT)) + 60 - sawtooth_center
# DVE float->uint8 conversion saturates [0,255] and rounds RNE (probed).
SCH_A = 4.0 * 1.4426950408889634 / TEMP   # 0.72135
SCH_B = 60.0 - 4.0 * 1.4426950408889634 * SHIFT - 0.229

# layout of the bf16 weight pack [64, BW]
_WQ0 = 0              # wq duplicated [64, 128]
_WK0 = 128            # wk duplicated [64, 128]
_WV0 = 256            # wv [64, 64]
BW = 320

# layout of the f32r pack [64, FW]
_ONES0 = 0            # ones column [64, 1]
_SEL0 = 1             # sel row at partition 32: [1, 66]
_WOE0 = 1 + MPAD      # woe [33 rows used, 66]
FW = 1 + MPAD + E + 2


def _bc_part(ap, n):
    """Broadcast a [x, ...] AP along a new leading partition dim of n."""
    return bass.AP(tensor=ap.tensor, offset=ap.offset, ap=[[0, n]] + list(ap.ap))


def _emit(tc, out_d, map_rows_d, mapT_d, obsT_d, wpb_d, wpf_d, c8_d, vec_d,
          dbg=None, exp_act_frac=0.5):
    nc = tc.nc
    with tc.tile_pool(name="consts", bufs=1) as consts, \
         tc.tile_pool(name="big", bufs=1) as big, \
         tc.tile_pool(name="sb_sm", bufs=3) as sb_sm, \
         tc.tile_pool(name="sb_pt", bufs=4) as sb_pt, \
         tc.tile_pool(name="ps", bufs=3, space="PSUM") as ps, \
         tc.tile_pool(name="ps_agg", bufs=2, space="PSUM") as ps_agg:

        # ---------------- constants ----------------
        wpb = consts.tile([E, BW], BF16)          # bf16 weights pack
        nc.sync.dma_start(wpb, wpb_d)
        wq2 = wpb[:, _WQ0:_WQ0 + 128]             # [64,128] wq|wq
        wk2 = wpb[:, _WK0:_WK0 + 128]             # [64,128] wk|wk
        wv = wpb[:, _WV0:_WV0 + E]                # [64,64]

        wpf = consts.tile([E, FW], F32R)
        nc.sync.dma_start(wpf, wpf_d)
        ones64 = wpf[:, _ONES0:_ONES0 + 1]
        sel66 = wpf[H:H + 1, _SEL0:_SEL0 + MPAD]  # row at partition 32
        woe = wpf[0:H + 1, _WOE0:_WOE0 + E + 2]   # [33,66]

        vecs = consts.tile([P, 3 * E + 1], F32)   # bo|gamma|beta|-shift
        nc.sync.dma_start(vecs, _bc_part(vec_d, P))
        bo_b = vecs[:, 0:E]
        ga_b = vecs[:, E:2 * E]
        be_b = vecs[:, 2 * E:3 * E]
        msh = vecs[:, 3 * E:3 * E + 1]            # -SHIFT bias column

        # ---------------- big arenas + input DMAs ----------------
        mapT = big.tile([E, NS], BF16)
        obsT = big.tile([E, NO], BF16)
        for lo, hi, t_, s_ in ((0, 512, mapT, mapT_d),
                               (0, 1024, obsT, obsT_d),
                               (512, 1024, mapT, mapT_d),
                               (1024, 2048, obsT, obsT_d),
                               (1024, 2048, mapT, mapT_d),
                               (2048, 4096, obsT, obsT_d),
                               (4096, 8192, obsT, obsT_d)):
            nc.sync.dma_start(t_[:, lo:hi], s_[:, lo:hi])
        map_rows = big.tile([P, NT, E], F32)
        nc.sync.dma_start(map_rows, map_rows_d.rearrange("(t p) e -> p t e", p=P))

        qT = big.tile([P, NS], F32R)              # map_q.T duplicated halves
        gmT = big.tile([H + 1, NS], F32R)         # [glu(map_v).T ; selfexp]
        okT = big.tile([P, NPAIR, P], F32R)       # obs_k.T parity-split
        gob8 = big.tile([P, NPAIR, 2, MPAD], FP8)  # glu(obs_v)|1|0 pairs
        ags = big.tile([H + 1, NS], F32R)         # [numer.T ; denom]
        map_pb = big.tile([P, NT, E], F32)        # map + bo
        out_pre = big.tile([P, NT, E], F32)
        out_all = big.tile([P, NT, E], F32)
        mvC = big.tile([P, NT, 2], F32)
        rstd = big.tile([P, NT], F32)

        # gob8 static columns: ones at h=32, zeros at h=33..65 (DMA from
        # the small HBM consts tensor, replicated via zero strides)
        gob8f = gob8.rearrange("p a b c -> p (a b) c")
        C8W = MPAD - H
        ones_src = bass.AP(tensor=c8_d.tensor, offset=c8_d.offset,
                           ap=[[C8W, P], [0, 2 * NPAIR], [0, 1]])
        nc.sync.dma_start(gob8f[:, :, H:H + 1], ones_src)
        zero_src = bass.AP(tensor=c8_d.tensor, offset=c8_d.offset + 1,
                           ap=[[C8W, P], [0, 2 * NPAIR], [1, MPAD - H - 1]])
        nc.sync.dma_start(gob8f[:, :, H + 1:MPAD], zero_src)

        # map + bo on gpsimd (all-SBUF)
        bo_rep = bass.AP(tensor=bo_b.tensor, offset=bo_b.offset,
                         ap=[list(bo_b.ap[0]), [0, NT], [1, E]])
        nc.gpsimd.tensor_tensor(out=map_pb, in0=map_rows, in1=bo_rep,
                                op=ALU.add)

        # ---------------- prologue pieces ----------------
        def map_chunk(c):
            """q (duplicated), selfexp, glu(map_v) for map cols [c*512, ..)."""
            sl = slice(c * GW, (c + 1) * GW)
            q_ps = ps.tile([P, 2, GW], F32, tag="st", name=f"qps{c}")
            nc.tensor.matmul(q_ps[:, 0, :], wq2, mapT[:, sl],
                             start=True, stop=True)
            nc.vector.tensor_copy(qT[:, sl], q_ps[:, 0, :])
            k_ps = ps.tile([P, 2, GW], F32, tag="st", name=f"kps{c}")
            nc.tensor.matmul(k_ps[:, 0, :], wk2, mapT[:, sl],
                             start=True, stop=True)
            qk = sb_sm.tile([E, GW], F32R, tag="qk", name=f"qk{c}")
            nc.vector.tensor_tensor(out=qk, in0=qT[0:E, sl],
                                    in1=k_ps[0:E, 0, :], op=ALU.mult)
            # self-score sum lands in the unused upper half of k_ps
            ss_ps = k_ps[0:1, 1, :]
            nc.tensor.matmul(ss_ps, ones64, qk, start=True, stop=True)
            nc.scalar.activation(gmT[H:H + 1, sl], ss_ps, AF.Exp,
                                 scale=1.0 / TEMP, bias=msh[0:1])
            v_ps = ps.tile([P, 2, GW], F32, tag="st", name=f"vps{c}")
            nc.tensor.matmul(v_ps[0:E, 0, :], wv, mapT[:, sl],
                             start=True, stop=True)
            th = sb_sm.tile([H, GW], F32, tag="th", name=f"th{c}")
            nc.scalar.activation(th, v_ps[H:E, 0, :], AF.Tanh, scale=0.5)
            nc.gpsimd.tensor_scalar(out=th, in0=th, scalar1=0.5, scalar2=0.5,
                                    op0=ALU.mult, op1=ALU.add)
            nc.vector.tensor_tensor(out=gmT[0:H, sl], in0=v_ps[0:H, 0, :],
                                    in1=th, op=ALU.mult)

        def obs_k_chunk2(c2, eng="v"):
            """okT parity-split fill for TWO obs chunks (one PSUM alloc).

            chunk c covers obs cols [c*512, ..) = blocks 4c..4c+3; even
            blocks land on partitions 0-63 of okT, odd blocks on 64-127
            (via the duplicated upper half of the wk2 projection, so no
            cross-partition movement is needed)."""
            k_ps = ps.tile([P, 2, GW], F32, tag="st", name=f"okps{c2}")
            for t in range(2):
                c = 2 * c2 + t
                sl = slice(c * GW, (c + 1) * GW)
                nc.tensor.matmul(k_ps[:, t, :], wk2, obsT[:, sl],
                                 start=True, stop=True)
            for t in range(2):
                c = 2 * c2 + t
                ev_in = k_ps[0:E, t, :].rearrange("p (b m) -> p b m",
                                                  b=2)[:, :, 0:P]
                ev_out = okT[0:E, 2 * c:2 * c + 2, :]
                od_in = k_ps[E:P, t, :].rearrange("p (b m) -> p b m",
                                                  b=2)[:, :, P:2 * P]
                od_out = okT[E:P, 2 * c:2 * c + 2, :]
                if eng == "v":
                    nc.vector.tensor_copy(ev_out, ev_in)
                    nc.vector.tensor_copy(od_out, od_in)
                else:
                    nc.scalar.copy(ev_out, ev_in)
                    nc.scalar.copy(od_out, od_in)

        def obs_v_batch2(c2, nb=16):
            """glu(obs_v) for nb consecutive obs blocks (one PSUM alloc)."""
            v_ps = ps.tile([P, 16, E], F32, tag="st", name=f"ovps{c2}")
            for b in range(nb):
                blk = c2 * 16 + b
                nc.tensor.matmul(v_ps[:, b, :],
                                 obsT[:, blk * P:(blk + 1) * P], wv,
                                 start=True, stop=True)
            tho = sb_sm.tile([P, 16, H], F32, tag="tho", name=f"tho{c2}")
            nc.scalar.activation(tho[:, 0:nb, :], v_ps[:, 0:nb, H:E],
                                 AF.Tanh, scale=0.5)
            nc.gpsimd.tensor_scalar(out=tho[:, 0:nb, :], in0=tho[:, 0:nb, :],
                                    scalar1=0.5, scalar2=0.5,
                                    op0=ALU.mult, op1=ALU.add)
            # blocks 16*c2.. -> pairs 8*c2.., t = parity
            og = gob8[:, 8 * c2:8 * c2 + nb // 2, :, 0:H]
            vi = v_ps[:, 0:nb, 0:H].rearrange("p (a b) h -> p a b h", b=2)
            ti = tho[:, 0:nb, :].rearrange("p (a b) h -> p a b h", b=2)
            nc.vector.tensor_tensor(out=og, in0=vi, in1=ti, op=ALU.mult)

        def agg_flush(g, agg, eng="v"):
            sl = slice(g * GW, (g + 1) * GW)
            if eng == "v":
                nc.vector.tensor_copy(ags[0:H + 1, sl], agg[0:H + 1, :])
            else:
                nc.scalar.copy(ags[0:H + 1, sl], agg[0:H + 1, :])

        # ---------------- epilogue ----------------
        def epi_half(half):
            """Batched epilogue for 8 map tiles: all PE matmuls first
            (into two grouped PSUM tiles, 512B-strided so every [128,66]
            output stays within one bank), then the elementwise chain
            pipelines across tiles on DVE/ACT without PE round-trips."""
            base = half * (NT // 2)
            uda = ps.tile([P, 8, P], F32, tag="st", name=f"uda{half}")
            gpa = ps.tile([P, 8, P], F32, tag="st", name=f"gpa{half}")
            for i in range(8):
                sl = slice((base + i) * P, (base + i + 1) * P)
                nc.tensor.matmul(uda[:, i, 0:E + 2], ags[:, sl], woe,
                                 start=True, stop=True)
                nc.tensor.matmul(gpa[:, i, 0:E + 2], gmT[:, sl], woe,
                                 start=True, stop=True)
            # evacuate PSUM immediately (uda/gpa sit in the ST rotation --
            # holding them through the elementwise chain would strangle the
            # main loop when this runs inside hp1)
            uds = sb_sm.tile([P, 8, E + 2], F32, tag="uds", name=f"uds{half}")
            nc.scalar.copy(uds, uda[:, :, 0:E + 2])
            gxs = sb_sm.tile([P, 8, E + 2], F32, tag="gxs", name=f"gxs{half}")
            nc.scalar.copy(gxs, gpa[:, :, 0:E + 2])
            rden = sb_sm.tile([P, 8], F32, tag="rden", name=f"rden{half}")
            nc.vector.reciprocal(rden, uds[:, :, E])
            for i in range(8):
                t = base + i
                ut = sb_sm.tile([P, E], F32, tag="ut", name=f"ut{t}")
                nc.vector.scalar_tensor_tensor(out=ut, in0=gxs[:, i, 0:E],
                                               scalar=gxs[:, i, E:E + 1],
                                               in1=uds[:, i, 0:E],
                                               op0=ALU.mult, op1=ALU.add)
                nc.vector.scalar_tensor_tensor(out=out_pre[:, t, :], in0=ut,
                                               scalar=rden[:, i:i + 1],
                                               in1=map_pb[:, t, :],
                                               op0=ALU.mult, op1=ALU.add)
                stats = sb_sm.tile([P, 6], F32, tag="stats", name=f"stats{t}")
                nc.vector.bn_stats(stats, out_pre[:, t, :])
                nc.vector.bn_aggr(mvC[:, t, :], stats)

        def epi_final(half, act_assist=True):
            tsl = slice(half * (NT // 2), (half + 1) * (NT // 2))
            w = NT // 2
            vpe = sb_sm.tile([P, w], F32, tag="vpe", name=f"vpe{half}")
            nc.vector.tensor_scalar_add(vpe, mvC[:, tsl, 1], EPS)
            c1 = sb_sm.tile([P, w], F32, tag="nc1", name=f"nc1{half}")
            nc.vector.tensor_scalar(out=c1, in0=vpe, scalar1=0.564185,
                                    scalar2=0.378467, op0=ALU.mult,
                                    op1=ALU.add)
            c2 = sb_sm.tile([P, w], F32, tag="nc2", name=f"nc2{half}")
            nc.vector.tensor_scalar(out=c2, in0=vpe, scalar1=0.288949,
                                    scalar2=0.791321, op0=ALU.mult,
                                    op1=ALU.add)
            nc.vector.tensor_tensor(out=c1, in0=c1, in1=c2, op=ALU.min)
            rs = rstd[:, tsl]
            nc.vector.reciprocal(rs, c1)
            for _ in range(3):
                nc.vector.tensor_tensor(out=c1, in0=rs, in1=rs, op=ALU.mult)
                nc.vector.tensor_tensor(out=c1, in0=c1, in1=vpe, op=ALU.mult)
                nc.vector.tensor_scalar(out=c1, in0=c1, scalar1=-0.5,
                                        scalar2=1.5, op0=ALU.mult,
                                        op1=ALU.add)
                nc.vector.tensor_tensor(out=rs, in0=rs, in1=c1, op=ALU.mult)
            for t in range(half * (NT // 2), (half + 1) * (NT // 2)):
                xn = sb_sm.tile([P, E], F32, tag="xn", name=f"xn{t}")
                if act_assist:
                    nmr = sb_sm.tile([P, 1], F32, tag="nmr", name=f"nmr{t}")
                    nc.vector.tensor_scalar(out=nmr, in0=mvC[:, t, 0:1],
                                            scalar1=rstd[:, t:t + 1],
                                            scalar2=-1.0, op0=ALU.mult,
                                            op1=ALU.mult)
                    nc.scalar.activation(xn, out_pre[:, t, :], AF.Identity,
                                         bias=nmr, scale=rstd[:, t:t + 1])
                else:
                    nc.vector.tensor_scalar(out=xn, in0=out_pre[:, t, :],
                                            scalar1=mvC[:, t, 0:1],
                                            scalar2=rstd[:, t:t + 1],
                                            op0=ALU.subtract, op1=ALU.mult)
                nc.gpsimd.tensor_tensor(out=xn, in0=xn, in1=ga_b, op=ALU.mult)
                nc.gpsimd.tensor_tensor(out=out_all[:, t, :], in0=xn,
                                        in1=be_b, op=ALU.add)
            od = out_d.rearrange("(t p) e -> p t e", p=P)
            for q in range(2):
                qsl = slice(half * (NT // 2) + q * (NT // 4),
                            half * (NT // 2) + (q + 1) * (NT // 4))
                nc.sync.dma_start(od[:, qsl, :], out_all[:, qsl, :])

        # ---------------- prologue head ----------------
        map_chunk(0)
        map_chunk(1)
        obs_k_chunk2(0)
        obs_v_batch2(0)
        map_chunk(2)
        map_chunk(3)

        # drip the remaining prologue into the first half-pass
        # drip schedule. IMPORTANT: obs_v_batch stays a single drip unit --
        # its PSUM tile comes from the shared rotating "st" tag, so the glu
        # must read it before the main loop's next st allocations wrap
        # around the pool and clobber the bank.
        drip = {}
        items = []
        for c2 in range(1, NO // GW // 2):
            items.append((4 * (c2 - 1), lambda c2=c2: obs_k_chunk2(c2)))
        for b2 in range(1, 4):
            items.append((7 * b2 - 3, lambda b2=b2: obs_v_batch2(b2)))
        items.sort(key=lambda x: x[0])
        used = set()
        for want, fn in items:
            pp = want
            while pp in used:
                pp += 1
            used.add(pp)
            drip.setdefault(pp, []).append(fn)

        # exp unit assignment: alternate engines per (pair, group); bias
        # toward ACT by granting it both groups every few pairs.
        def exp_unit(st_t, pt_t, eng):
            if eng == "a":
                nc.scalar.activation(pt_t, st_t, AF.Exp,
                                     scale=1.0 / TEMP, bias=msh)
            else:
                nc.vector.tensor_scalar(out=pt_t.bitcast(U8), in0=st_t,
                                        scalar1=SCH_A, scalar2=SCH_B,
                                        op0=ALU.mult, op1=ALU.add)

        # ---------------- main loop: 2 half-passes x 32 pairs ----------
        # Software-pipelined by one pair: the PV for pair p-1 is issued to
        # the PE AFTER pair p's ST matmuls, so by the time the PE FIFO
        # reaches it, exp(p-1) has long finished -- no head-of-line stall.
        for hp in range(2):
            agg0 = ps_agg.tile([MPAD, GW], F32, tag="agg", name=f"agg{hp}_0")
            agg1 = ps_agg.tile([MPAD, GW], F32, tag="agg", name=f"agg{hp}_1")
            g0 = 2 * hp
            g1 = 2 * hp + 1
            s0 = slice(g0 * GW, (g0 + 1) * GW)
            s1 = slice(g1 * GW, (g1 + 1) * GW)
            nc.tensor.matmul(agg0, sel66, gmT[H:H + 1, s0],
                             start=True, stop=False)
            nc.tensor.matmul(agg1, sel66, gmT[H:H + 1, s1],
                             start=True, stop=False)
            prev_pt = None
            for pp in range(NPAIR):
                st0 = ps.tile([P, 2, GW], F32, tag="st", name=f"st{hp}_{pp}_0")
                st1 = ps.tile([P, 2, GW], F32, tag="st", name=f"st{hp}_{pp}_1")
                ko_lo = okT[0:E, pp, :]
                ko_hi = okT[E:P, pp, :]
                nc.tensor.matmul(st0[:, 0, :], ko_lo, qT[0:E, s0],
                                 start=True, stop=True)
                nc.tensor.matmul(st0[:, 1, :], ko_hi, qT[E:P, s0],
                                 start=True, stop=True)
                nc.tensor.matmul(st1[:, 0, :], ko_lo, qT[0:E, s1],
                                 start=True, stop=True)
                nc.tensor.matmul(st1[:, 1, :], ko_hi, qT[E:P, s1],
                                 start=True, stop=True)
                if prev_pt is not None:
                    qq, qt0, qt1 = prev_pt
                    go = gob8[:, qq, :, :]
                    nc.tensor.matmul(agg0, go, qt0, start=False, stop=False,
                                     perf_mode=DR)
                    nc.tensor.matmul(agg1, go, qt1, start=False, stop=False,
                                     perf_mode=DR)
                pt0 = sb_pt.tile([P, 2, GW], FP8E5, tag="pt",
                                 name=f"pt{hp}_{pp}_0")
                pt1 = sb_pt.tile([P, 2, GW], FP8E5, tag="pt",
                                 name=f"pt{hp}_{pp}_1")
                # exp split: in hp0 DVE also carries the drip (casts/glu),
                # so ACT takes both groups every 4th pair; in hp1 the
                # engines are evenly loaded, so strict 1:1.
                bonus = (pp % 4 == 3) if hp == 0 else False
                exp_unit(st0, pt0, "a")
                exp_unit(st1, pt1, "a" if bonus else "v")
                prev_pt = (pp, pt0, pt1)
                if hp == 0:
                    for fn in drip.get(pp, ()):
                        fn()
                else:
                    # hp0's ags columns are final: run its epilogue during
                    # hp1 (batched -- only two extra PSUM allocs total)
                    if pp == 6:
                        epi_half(0)
                    elif pp == 16:
                        epi_final(0)
            qq, qt0, qt1 = prev_pt
            go = gob8[:, qq, :, :]
            nc.tensor.matmul(agg0, go, qt0, start=False, stop=True,
                             perf_mode=DR)
            nc.tensor.matmul(agg1, go, qt1, start=False, stop=True,
                             perf_mode=DR)
            agg_flush(g0, agg0, eng="v")
            agg_flush(g1, agg1, eng="a")

        # ---------------- epilogue (half 0 ran during hp1) ----------
        epi_half(1)
        epi_final(1)

        if dbg is not None:
            nc.sync.dma_start(dbg["qT"], qT)
            nc.sync.dma_start(dbg["gmT"], gmT)
            nc.sync.dma_start(dbg["ags"], ags)
            nc.sync.dma_start(dbg["okT"], okT.rearrange("p a b -> p (a b)"))
            nc.sync.dma_start(dbg["gob8"],
                              gob8.rearrange("p a b c -> p (a b c)"))
            nc.sync.dma_start(dbg["out_pre"],
                              out_pre.rearrange("p a b -> p (a b)"))
            nc.sync.dma_start(dbg["mvC"], mvC.rearrange("p a b -> p (a b)"))


_CACHED = None


def _build(debug=False):
    global _CACHED
    if _CACHED is not None and not debug:
        return _CACHED
    nc = bacc.Bacc("TRN2", target_bir_lowering=False, debug=False)

    def din(name, shape, dt=F32):
        return nc.dram_tensor(name, shape, dt, kind="ExternalInput").ap()

    map_rows_d = din("map_rows", [NS, E])
    mapT_d = din("mapT", [E, NS], BF16)
    obsT_d = din("obsT", [E, NO], BF16)
    wpb_d = din("wpb", [E, BW], BF16)
    wpf_d = din("wpf", [E, FW], F32R)
    c8_d = din("c8", [P, MPAD - H], FP8)
    vec_d = din("vpack", [3 * E + 1])
    out_d = nc.dram_tensor("out", [NS, E], F32, kind="ExternalOutput").ap()

    dbg = None
    if debug:
        def dout(name, shape, dt=F32):
            return nc.dram_tensor(name, shape, dt, kind="ExternalOutput").ap()
        dbg = {
            "qT": dout("dbg_qT", [P, NS], F32R),
            "gmT": dout("dbg_gmT", [H + 1, NS], F32R),
            "ags": dout("dbg_ags", [H + 1, NS], F32R),
            "okT": dout("dbg_okT", [P, NPAIR * P], F32R),
            "gob8": dout("dbg_gob8", [P, NPAIR * 2 * MPAD], FP8),
            "out_pre": dout("dbg_out_pre", [P, NT * E]),
            "mvC": dout("dbg_mvC", [P, NT * 2]),
        }

    with tile.TileContext(nc) as tc:
        _emit(tc, out_d, map_rows_d, mapT_d, obsT_d, wpb_d, wpf_d, c8_d,
              vec_d, dbg=dbg)
    nc.compile()
    if not debug:
        _CACHED = nc
    return nc


def _prep_in_maps(map_code, obs_code, Wq, Wk, Wv, Wo, bo, gamma, beta):
    f = np.float32
    map_code = np.ascontiguousarray(np.asarray(map_code, dtype=f))
    obs_code = np.asarray(obs_code, dtype=f)

    bf16_np = mybir.dt.np(BF16)
    fp8_np = mybir.dt.np(FP8)

    def to_bf16(x):
        return np.ascontiguousarray(np.asarray(x, dtype=f).astype(bf16_np))

    obsT = np.ascontiguousarray(obs_code.T)

    wq2 = np.concatenate([np.asarray(Wq, f), np.asarray(Wq, f)], axis=1)
    wk2 = np.concatenate([np.asarray(Wk, f), np.asarray(Wk, f)], axis=1)
    woe = np.zeros((E, E + 2), dtype=f)
    woe[0:H, 0:E] = np.asarray(Wo, dtype=f)
    woe[H, E] = 1.0
    wpb = np.zeros((E, BW), dtype=f)
    wpb[:, _WQ0:_WQ0 + 128] = wq2
    wpb[:, _WK0:_WK0 + 128] = wk2
    wpb[:, _WV0:_WV0 + E] = np.asarray(Wv, f)

    wpf = np.zeros((E, FW), dtype=f)
    wpf[:, _ONES0] = 1.0
    wpf[H, _SEL0 + H] = 1.0   # sel row: 1.0 at position 32 (partition 32)
    wpf[:, _WOE0:_WOE0 + E + 2] = woe

    c8 = np.zeros((P, MPAD - H), dtype=fp8_np)
    c8[:, 0] = 1.0

    vpack = np.concatenate([
        np.asarray(bo, dtype=f), np.asarray(gamma, dtype=f),
        np.asarray(beta, dtype=f), np.full((1,), -SHIFT, dtype=f),
    ])
    shared = {
        "obsT": to_bf16(obsT),
        "wpb": to_bf16(wpb),
        "wpf": np.ascontiguousarray(wpf),
        "c8": np.ascontiguousarray(c8),
        "vpack": np.ascontiguousarray(vpack),
    }
    in_maps = []
    for i in range(NCORES):
        shard = map_code[i * NS:(i + 1) * NS]
        m = dict(shared)
        m["map_rows"] = shard
        m["mapT"] = to_bf16(np.ascontiguousarray(shard.T))
        in_maps.append(m)
    return in_maps


def run(trace=False, **inputs):
    nc = _build()
    in_maps = _prep_in_maps(**inputs)
    res = run_bass_kernel_spmd(nc, in_maps, list(range(NCORES)), trace=trace)
    out = np.concatenate([res.results[i]["out"] for i in range(NCORES)], axis=0)
    return out, res


def kernel(**inputs):
    out, _ = run(trace=False, **inputs)
    return out


# revision 31
# speedup vs baseline: 1.2836x; 1.1816x over previous
"""Trainium2 Bass kernel for nn_Attention_5815385719367 (gnn_message_passing).

Computation (see reference):
  map_q/k/v = map_code @ Wq/Wk/Wv ; obs_k/v = obs_code @ Wk/Wv
  scores    = [sum(q*k,-1) | q @ obs_k.T] / 8
  w         = softmax(scores)
  agg       = w[:, :1]*glu(map_v) + w[:, 1:] @ glu(obs_v)
  out       = LN(agg @ Wo + bo + map_code) * gamma + beta

Sharding: data-parallel over N_map rows (2048 rows/core x 8 cores);
obs_code and weights replicated. No collectives.

v2 design notes (per core):
  - scores computed TRANSPOSED in PSUM: ST[obs=128, map] via PE ROW-TILED
    pairs: even obs block's k.T sits on SBUF partitions 0-63 (PE tile
    (0,0)), odd block's on partitions 64-127 (tile (64,0)); the two
    256-cycle streams run CONCURRENTLY in different PE row groups, so
    ST costs ~0.5 cyc/col. qT and okT are produced pre-duplicated /
    parity-split by projecting with host-duplicated weights
    ([64,128] wq|wq and wk|wk), so no cross-partition copies exist.
  - softmax exp is the hard wall (1 elem/lane/cycle on ACT): split it
    between ACT (direct exp -> fp8e4m3, logits shifted by -1 so
    exp <= ~110 < 240 = trn-e4m3 max) and DVE (Schraudolph: one
    mult-add tensor_scalar into uint8, whose bit pattern IS the
    e4m3 log-domain approximation; errors ~+-5% are noise-like and
    average out over 8k obs).
  - PV runs fp8 DoubleRow over block PAIRS: stationary
    gob8[128, 2, 66] = glu(obs_v)|ones|zero-pad for (even, odd)
    blocks, moving pt8[128, 2, 512] = exp'd scores; contraction is
    256 at 0.5 cyc/col. M padded 33->66 keeps col tiling off
    (DoubleRow is incompatible with column tiling). The ones column
    accumulates the softmax denominator for free.
  - self-attention term folded in after Wo (as v1): agg row 32 is
    seeded with selfexp, epilogue adds selfexp*(glu(map_v)@Wo) and
    divides by the denominator; the -1 logit shift cancels exactly.
  - projections in bf16 (inputs shipped bf16), epilogue Wo in bf16.
  - GPSIMD does SBUF-side elementwise work (sigmoid affine steps,
    map+bo, gamma/beta) since it cannot touch PSUM.
"""

import numpy as np

import concourse.bass as bass
import concourse.bacc as bacc
import concourse.tile as tile
from concourse import mybir
from concourse.bass_utils import run_bass_kernel_spmd

NCORES = 8
NM, NO, E = 16384, 8192, 64
NS = NM // NCORES            # 2048 map rows per core
H = E // 2                   # 32
TEMP = 8.0
EPS = 1e-6
P = 128
NT = NS // P                 # 16 row tiles per core
GW = 512                     # map group width (psum bank)
NPAIR = NO // 256            # 32 obs block-pairs
SHIFT = -2.0                 # logit shift: exp(l - SHIFT), cancels in ratio.
                             # Logits empirically span [-8.1, 8.32]; e5m2's
                             # 22-e-fold range with shift -2 covers all of it
                             # (max exp ~ e^10.3 = 3e4 < 57344) with no
                             # clipping at either end.
MPAD = 80                    # padded PV output partitions (33 real; %16 keeps the DoubleRow ldweights step legal, >64 keeps column tiling off)

F32 = mybir.dt.float32
F32R = mybir.dt.float32r
BF16 = mybir.dt.bfloat16
FP8 = mybir.dt.float8e4
FP8E5 = mybir.dt.float8e5
U8 = mybir.dt.uint8
AF = mybir.ActivationFunctionType
ALU = mybir.AluOpType
DR = mybir.MatmulPerfMode.DoubleRow

# Schraudolph constants for uint8 e5m2 log-domain exp of RAW score s:
#   i = 4*log2(exp(s/8 - SHIFT)) + 60 - sawtooth_center
# DVE float->uint8 conversion saturates [0,255] and rounds RNE (probed).
SCH_A = 4.0 * 1.4426950408889634 / TEMP   # 0.72135
SCH_B = 60.0 - 4.0 * 1.4426950408889634 * SHIFT - 0.229

# layout of the bf16 weight pack [64, BW]
_WQ0 = 0              # wq duplicated [64, 128]
_WK0 = 128            # wk duplicated [64, 128]
_WV0 = 256            # wv [64, 64]
BW = 320

# layout of the f32r pack [64, FW]
_ONES0 = 0            # ones column [64, 1]
_SEL0 = 1             # sel row at partition 32: [1, 66]
_WOE0 = 1 + MPAD      # woe [33 rows used, 66]
FW = 1 + MPAD + E + 2


def _bc_part(ap, n):
    """Broadcast a [x, ...] AP along a new leading partition dim of n."""
    return bass.AP(tensor=ap.tensor, offset=ap.offset, ap=[[0, n]] + list(ap.ap))


def _emit(tc, out_d, map_rows_d, mapT_d, obsT_d, wpb_d, wpf_d, c8_d, vec_d,
          dbg=None, exp_act_frac=0.5):
    nc = tc.nc
    with tc.tile_pool(name="consts", bufs=1) as consts, \
         tc.tile_pool(name="big", bufs=1) as big, \
         tc.tile_pool(name="sb_sm", bufs=3) as sb_sm, \
         tc.tile_pool(name="sb_pt", bufs=4) as sb_pt, \
         tc.tile_pool(name="ps", bufs=3, space="PSUM") as ps, \
         tc.tile_pool(name="ps_agg", bufs=2, space="PSUM") as ps_agg:

        # ---------------- constants ----------------
        wpb = consts.tile([E, BW], BF16)          # bf16 weights pack
        nc.sync.dma_start(wpb, wpb_d)
        wq2 = wpb[:, _WQ0:_WQ0 + 128]             # [64,128] wq|wq
        wk2 = wpb[:, _WK0:_WK0 + 128]             # [64,128] wk|wk
        wv = wpb[:, _WV0:_WV0 + E]                # [64,64]

        wpf = consts.tile([E, FW], F32R)
        nc.sync.dma_start(wpf, wpf_d)
        ones64 = wpf[:, _ONES0:_ONES0 + 1]
        sel66 = wpf[H:H + 1, _SEL0:_SEL0 + MPAD]  # row at partition 32
        woe = wpf[0:H + 1, _WOE0:_WOE0 + E + 2]   # [33,66]

        vecs = consts.tile([P, 3 * E + 1], F32)   # bo|gamma|beta|-shift
        nc.sync.dma_start(vecs, _bc_part(vec_d, P))
        bo_b = vecs[:, 0:E]
        ga_b = vecs[:, E:2 * E]
        be_b = vecs[:, 2 * E:3 * E]
        msh = vecs[:, 3 * E:3 * E + 1]            # -SHIFT bias column

        # ---------------- big arenas + input DMAs ----------------
        mapT = big.tile([E, NS], BF16)
        obsT = big.tile([E, NO], BF16)
        for lo, hi, t_, s_ in ((0, 512, mapT, mapT_d),
                               (0, 1024, obsT, obsT_d),
                               (512, 1024, mapT, mapT_d),
                               (1024, 2048, obsT, obsT_d),
                               (1024, 2048, mapT, mapT_d),
                               (2048, 4096, obsT, obsT_d),
                               (4096, 8192, obsT, obsT_d)):
            nc.sync.dma_start(t_[:, lo:hi], s_[:, lo:hi])
        map_rows = big.tile([P, NT, E], F32)
        nc.sync.dma_start(map_rows, map_rows_d.rearrange("(t p) e -> p t e", p=P))

        qT = big.tile([P, NS], F32R)              # map_q.T duplicated halves
        gmT = big.tile([H + 1, NS], F32R)         # [glu(map_v).T ; selfexp]
        okT = big.tile([P, NPAIR, P], F32R)       # obs_k.T parity-split
        gob8 = big.tile([P, NPAIR, 2, MPAD], FP8)  # glu(obs_v)|1|0 pairs
        ags = big.tile([H + 1, NS], F32R)         # [numer.T ; denom]
        map_pb = big.tile([P, NT, E], F32)        # map + bo
        out_pre = big.tile([P, NT, E], F32)
        out_all = big.tile([P, NT, E], F32)
        mvC = big.tile([P, NT, 2], F32)
        rstd = big.tile([P, NT], F32)

        # gob8 static columns: ones at h=32, zeros at h=33..65 (DMA from
        # the small HBM consts tensor, replicated via zero strides)
        gob8f = gob8.rearrange("p a b c -> p (a b) c")
        C8W = MPAD - H
        ones_src = bass.AP(tensor=c8_d.tensor, offset=c8_d.offset,
                           ap=[[C8W, P], [0, 2 * NPAIR], [0, 1]])
        nc.sync.dma_start(gob8f[:, :, H:H + 1], ones_src)
        zero_src = bass.AP(tensor=c8_d.tensor, offset=c8_d.offset + 1,
                           ap=[[C8W, P], [0, 2 * NPAIR], [1, MPAD - H - 1]])
        nc.sync.dma_start(gob8f[:, :, H + 1:MPAD], zero_src)

        # map + bo on gpsimd (all-SBUF)
        bo_rep = bass.AP(tensor=bo_b.tensor, offset=bo_b.offset,
                         ap=[list(bo_b.ap[0]), [0, NT], [1, E]])
        nc.gpsimd.tensor_tensor(out=map_pb, in0=map_rows, in1=bo_rep,
                                op=ALU.add)

        # ---------------- prologue pieces ----------------
        def map_chunk(c):
            """q (duplicated), selfexp, glu(map_v) for map cols [c*512, ..)."""
            sl = slice(c * GW, (c + 1) * GW)
            q_ps = ps.tile([P, 2, GW], F32, tag="st", name=f"qps{c}")
            nc.tensor.matmul(q_ps[:, 0, :], wq2, mapT[:, sl],
                             start=True, stop=True)
            nc.vector.tensor_copy(qT[:, sl], q_ps[:, 0, :])
            k_ps = ps.tile([P, 2, GW], F32, tag="st", name=f"kps{c}")
            nc.tensor.matmul(k_ps[:, 0, :], wk2, mapT[:, sl],
                             start=True, stop=True)
            qk = sb_sm.tile([E, GW], F32R, tag="qk", name=f"qk{c}")
            nc.vector.tensor_tensor(out=qk, in0=qT[0:E, sl],
                                    in1=k_ps[0:E, 0, :], op=ALU.mult)
            # self-score sum lands in the unused upper half of k_ps
            ss_ps = k_ps[0:1, 1, :]
            nc.tensor.matmul(ss_ps, ones64, qk, start=True, stop=True)
            nc.scalar.activation(gmT[H:H + 1, sl], ss_ps, AF.Exp,
                                 scale=1.0 / TEMP, bias=msh[0:1])
            v_ps = ps.tile([P, 2, GW], F32, tag="st", name=f"vps{c}")
            nc.tensor.matmul(v_ps[0:E, 0, :], wv, mapT[:, sl],
                             start=True, stop=True)
            th = sb_sm.tile([H, GW], F32, tag="th", name=f"th{c}")
            nc.scalar.activation(th, v_ps[H:E, 0, :], AF.Tanh, scale=0.5)
            nc.gpsimd.tensor_scalar(out=th, in0=th, scalar1=0.5, scalar2=0.5,
                                    op0=ALU.mult, op1=ALU.add)
            nc.vector.tensor_tensor(out=gmT[0:H, sl], in0=v_ps[0:H, 0, :],
                                    in1=th, op=ALU.mult)

        def obs_k_chunk2(c2, eng="v"):
            """okT parity-split fill for TWO obs chunks (one PSUM alloc).

            chunk c covers obs cols [c*512, ..) = blocks 4c..4c+3; even
            blocks land on partitions 0-63 of okT, odd blocks on 64-127
            (via the duplicated upper half of the wk2 projection, so no
            cross-partition movement is needed)."""
            k_ps = ps.tile([P, 2, GW], F32, tag="st", name=f"okps{c2}")
            for t in range(2):
                c = 2 * c2 + t
                sl = slice(c * GW, (c + 1) * GW)
                nc.tensor.matmul(k_ps[:, t, :], wk2, obsT[:, sl],
                                 start=True, stop=True)
            for t in range(2):
                c = 2 * c2 + t
                ev_in = k_ps[0:E, t, :].rearrange("p (b m) -> p b m",
                                                  b=2)[:, :, 0:P]
                ev_out = okT[0:E, 2 * c:2 * c + 2, :]
                od_in = k_ps[E:P, t, :].rearrange("p (b m) -> p b m",
                                                  b=2)[:, :, P:2 * P]
                od_out = okT[E:P, 2 * c:2 * c + 2, :]
                if eng == "v":
                    nc.vector.tensor_copy(ev_out, ev_in)
                    nc.vector.tensor_copy(od_out, od_in)
                else:
                    nc.scalar.copy(ev_out, ev_in)
                    nc.scalar.copy(od_out, od_in)

        def obs_v_batch2(c2, nb=16):
            """glu(obs_v) for nb consecutive obs blocks (one PSUM alloc)."""
            v_ps = ps.tile([P, 16, E], F32, tag="st", name=f"ovps{c2}")
            for b in range(nb):
                blk = c2 * 16 + b
                nc.tensor.matmul(v_ps[:, b, :],
                                 obsT[:, blk * P:(blk + 1) * P], wv,
                                 start=True, stop=True)
            tho = sb_sm.tile([P, 16, H], F32, tag="tho", name=f"tho{c2}")
            nc.scalar.activation(tho[:, 0:nb, :], v_ps[:, 0:nb, H:E],
                                 AF.Tanh, scale=0.5)
            nc.gpsimd.tensor_scalar(out=tho[:, 0:nb, :], in0=tho[:, 0:nb, :],
                                    scalar1=0.5, scalar2=0.5,
                                    op0=ALU.mult, op1=ALU.add)
            # blocks 16*c2.. -> pairs 8*c2.., t = parity
            og = gob8[:, 8 * c2:8 * c2 + nb // 2, :, 0:H]
            vi = v_ps[:, 0:nb, 0:H].rearrange("p (a b) h -> p a b h", b=2)
            ti = tho[:, 0:nb, :].rearrange("p (a b) h -> p a b h", b=2)
            nc.vector.tensor_tensor(out=og, in0=vi, in1=ti, op=ALU.mult)

        def agg_flush(g, agg, eng="v"):
            sl = slice(g * GW, (g + 1) * GW)
            if eng == "v":
                nc.vector.tensor_copy(ags[0:H + 1, sl], agg[0:H + 1, :])
            else:
                nc.scalar.copy(ags[0:H + 1, sl], agg[0:H + 1, :])

        # ---------------- epilogue ----------------
        def epi_half(half):
            """Batched epilogue for 8 map tiles: all PE matmuls first
            (into two grouped PSUM tiles, 512B-strided so every [128,66]
            output stays within one bank), then the elementwise chain
            pipelines across tiles on DVE/ACT without PE round-trips."""
            base = half * (NT // 2)
            uda = ps.tile([P, 8, P], F32, tag="st", name=f"uda{half}")
            gpa = ps.tile([P, 8, P], F32, tag="st", name=f"gpa{half}")
            for i in range(8):
                sl = slice((base + i) * P, (base + i + 1) * P)
                nc.tensor.matmul(uda[:, i, 0:E + 2], ags[:, sl], woe,
                                 start=True, stop=True)
                nc.tensor.matmul(gpa[:, i, 0:E + 2], gmT[:, sl], woe,
                                 start=True, stop=True)
            # evacuate PSUM immediately (uda/gpa sit in the ST rotation --
            # holding them through the elementwise chain would strangle the
            # main loop when this runs inside hp1)
            uds = sb_sm.tile([P, 8, E + 2], F32, tag="uds", name=f"uds{half}")
            nc.vector.tensor_copy(uds, uda[:, :, 0:E + 2])
            gxs = sb_sm.tile([P, 8, E + 2], F32, tag="gxs", name=f"gxs{half}")
            nc.scalar.copy(gxs, gpa[:, :, 0:E + 2])
            rden = sb_sm.tile([P, 8], F32, tag="rden", name=f"rden{half}")
            nc.vector.reciprocal(rden, uds[:, :, E])
            for i in range(8):
                t = base + i
                ut = sb_sm.tile([P, E], F32, tag="ut", name=f"ut{t}")
                nc.vector.scalar_tensor_tensor(out=ut, in0=gxs[:, i, 0:E],
                                               scalar=gxs[:, i, E:E + 1],
                                               in1=uds[:, i, 0:E],
                                               op0=ALU.mult, op1=ALU.add)
                nc.vector.scalar_tensor_tensor(out=out_pre[:, t, :], in0=ut,
                                               scalar=rden[:, i:i + 1],
                                               in1=map_pb[:, t, :],
                                               op0=ALU.mult, op1=ALU.add)
                stats = sb_sm.tile([P, 6], F32, tag="stats", name=f"stats{t}")
                nc.vector.bn_stats(stats, out_pre[:, t, :])
                nc.vector.bn_aggr(mvC[:, t, :], stats)

        def epi_final(half, act_assist=True):
            tsl = slice(half * (NT // 2), (half + 1) * (NT // 2))
            w = NT // 2
            vpe = sb_sm.tile([P, w], F32, tag="vpe", name=f"vpe{half}")
            nc.vector.tensor_scalar_add(vpe, mvC[:, tsl, 1], EPS)
            c1 = sb_sm.tile([P, w], F32, tag="nc1", name=f"nc1{half}")
            nc.vector.tensor_scalar(out=c1, in0=vpe, scalar1=0.564185,
                                    scalar2=0.378467, op0=ALU.mult,
                                    op1=ALU.add)
            c2 = sb_sm.tile([P, w], F32, tag="nc2", name=f"nc2{half}")
            nc.vector.tensor_scalar(out=c2, in0=vpe, scalar1=0.288949,
                                    scalar2=0.791321, op0=ALU.mult,
                                    op1=ALU.add)
            nc.vector.tensor_tensor(out=c1, in0=c1, in1=c2, op=ALU.min)
            rs = rstd[:, tsl]
            nc.vector.reciprocal(rs, c1)
            for _ in range(3):
                nc.vector.tensor_tensor(out=c1, in0=rs, in1=rs, op=ALU.mult)
                nc.vector.tensor_tensor(out=c1, in0=c1, in1=vpe, op=ALU.mult)
                nc.vector.tensor_scalar(out=c1, in0=c1, scalar1=-0.5,
                                        scalar2=1.5, op0=ALU.mult,
                                        op1=ALU.add)
                nc.vector.tensor_tensor(out=rs, in0=rs, in1=c1, op=ALU.mult)
            for t in range(half * (NT // 2), (half + 1) * (NT // 2)):
                xn = sb_sm.tile([P, E], F32, tag="xn", name=f"xn{t}")
                if act_assist:
                    nmr = sb_sm.tile([P, 1], F32, tag="nmr", name=f"nmr{t}")
                    nc.vector.tensor_scalar(out=nmr, in0=mvC[:, t, 0:1],
                                            scalar1=rstd[:, t:t + 1],
                                            scalar2=-1.0, op0=ALU.mult,
                                            op1=ALU.mult)
                    nc.scalar.activation(xn, out_pre[:, t, :], AF.Identity,
                                         bias=nmr, scale=rstd[:, t:t + 1])
                else:
                    nc.vector.tensor_scalar(out=xn, in0=out_pre[:, t, :],
                                            scalar1=mvC[:, t, 0:1],
                                            scalar2=rstd[:, t:t + 1],
                                            op0=ALU.subtract, op1=ALU.mult)
                nc.gpsimd.tensor_tensor(out=xn, in0=xn, in1=ga_b, op=ALU.mult)
                nc.gpsimd.tensor_tensor(out=out_all[:, t, :], in0=xn,
                                        in1=be_b, op=ALU.add)
            od = out_d.rearrange("(t p) e -> p t e", p=P)
            for q in range(2):
                qsl = slice(half * (NT // 2) + q * (NT // 4),
                            half * (NT // 2) + (q + 1) * (NT // 4))
                nc.sync.dma_start(od[:, qsl, :], out_all[:, qsl, :])

        # ---------------- prologue head ----------------
        map_chunk(0)
        map_chunk(1)
        obs_k_chunk2(0)
        obs_v_batch2(0)
        map_chunk(2)
        map_chunk(3)

        # drip the remaining prologue into the first half-pass
        # drip schedule. IMPORTANT: obs_v_batch stays a single drip unit --
        # its PSUM tile comes from the shared rotating "st" tag, so the glu
        # must read it before the main loop's next st allocations wrap
        # around the pool and clobber the bank.
        drip = {}
        items = []
        for c2 in range(1, NO // GW // 2):
            items.append((4 * (c2 - 1), lambda c2=c2: obs_k_chunk2(c2)))
        for b2 in range(1, 4):
            items.append((7 * b2 - 3, lambda b2=b2: obs_v_batch2(b2)))
        items.sort(key=lambda x: x[0])
        used = set()
        for want, fn in items:
            pp = want
            while pp in used:
                pp += 1
            used.add(pp)
            drip.setdefault(pp, []).append(fn)

        # exp unit assignment: alternate engines per (pair, group); bias
        # toward ACT by granting it both groups every few pairs.
        def exp_unit(st_t, pt_t, eng):
            if eng == "a":
                nc.scalar.activation(pt_t, st_t, AF.Exp,
                                     scale=1.0 / TEMP, bias=msh)
            else:
                nc.vector.tensor_scalar(out=pt_t.bitcast(U8), in0=st_t,
                                        scalar1=SCH_A, scalar2=SCH_B,
                                        op0=ALU.mult, op1=ALU.add)

        # ---------------- main loop: 2 half-passes x 32 pairs ----------
        # Software-pipelined by one pair: the PV for pair p-1 is issued to
        # the PE AFTER pair p's ST matmuls, so by the time the PE FIFO
        # reaches it, exp(p-1) has long finished -- no head-of-line stall.
        for hp in range(2):
            agg0 = ps_agg.tile([MPAD, GW], F32, tag="agg", name=f"agg{hp}_0")
            agg1 = ps_agg.tile([MPAD, GW], F32, tag="agg", name=f"agg{hp}_1")
            g0 = 2 * hp
            g1 = 2 * hp + 1
            s0 = slice(g0 * GW, (g0 + 1) * GW)
            s1 = slice(g1 * GW, (g1 + 1) * GW)
            nc.tensor.matmul(agg0, sel66, gmT[H:H + 1, s0],
                             start=True, stop=False)
            nc.tensor.matmul(agg1, sel66, gmT[H:H + 1, s1],
                             start=True, stop=False)
            prev_pt = None
            for pp in range(NPAIR):
                st0 = ps.tile([P, 2, GW], F32, tag="st", name=f"st{hp}_{pp}_0")
                st1 = ps.tile([P, 2, GW], F32, tag="st", name=f"st{hp}_{pp}_1")
                ko_lo = okT[0:E, pp, :]
                ko_hi = okT[E:P, pp, :]
                nc.tensor.matmul(st0[:, 0, :], ko_lo, qT[0:E, s0],
                                 start=True, stop=True)
                nc.tensor.matmul(st0[:, 1, :], ko_hi, qT[E:P, s0],
                                 start=True, stop=True)
                nc.tensor.matmul(st1[:, 0, :], ko_lo, qT[0:E, s1],
                                 start=True, stop=True)
                nc.tensor.matmul(st1[:, 1, :], ko_hi, qT[E:P, s1],
                                 start=True, stop=True)
                if prev_pt is not None:
                    qq, qt0, qt1 = prev_pt
                    go = gob8[:, qq, :, :]
                    nc.tensor.matmul(agg0, go, qt0, start=False, stop=False,
                                     perf_mode=DR)
                    nc.tensor.matmul(agg1, go, qt1, start=False, stop=False,
                                     perf_mode=DR)
                pt0 = sb_pt.tile([P, 2, GW], FP8E5, tag="pt",
                                 name=f"pt{hp}_{pp}_0")
                pt1 = sb_pt.tile([P, 2, GW], FP8E5, tag="pt",
                                 name=f"pt{hp}_{pp}_1")
                # exp split: in hp0 DVE also carries the drip (casts/glu),
                # so ACT takes both groups every 4th pair; in hp1 the
                # engines are evenly loaded, so strict 1:1.
                bonus = (pp % 4 == 3) if hp == 0 else False
                exp_unit(st0, pt0, "a")
                exp_unit(st1, pt1, "a" if bonus else "v")
                prev_pt = (pp, pt0, pt1)
                if hp == 0:
                    for fn in drip.get(pp, ()):
                        fn()
                else:
                    # hp0's ags columns are final: run its epilogue during
                    # hp1 (batched -- only two extra PSUM allocs total)
                    if pp == 6:
                        epi_half(0)
                    elif pp == 16:
                        epi_final(0)
            qq, qt0, qt1 = prev_pt
            go = gob8[:, qq, :, :]
            nc.tensor.matmul(agg0, go, qt0, start=False, stop=True,
                             perf_mode=DR)
            nc.tensor.matmul(agg1, go, qt1, start=False, stop=True,
                             perf_mode=DR)
            agg_flush(g0, agg0, eng="v")
            agg_flush(g1, agg1, eng="a")

        # ---------------- epilogue (half 0 ran during hp1) ----------
        epi_half(1)
        epi_final(1)

        if dbg is not None:
            nc.sync.dma_start(dbg["qT"], qT)
            nc.sync.dma_start(dbg["gmT"], gmT)
            nc.sync.dma_start(dbg["ags"], ags)
            nc.sync.dma_start(dbg["okT"], okT.rearrange("p a b -> p (a b)"))
            nc.sync.dma_start(dbg["gob8"],
                              gob8.rearrange("p a b c -> p (a b c)"))
            nc.sync.dma_start(dbg["out_pre"],
                              out_pre.rearrange("p a b -> p (a b)"))
            nc.sync.dma_start(dbg["mvC"], mvC.rearrange("p a b -> p (a b)"))


_CACHED = None


def _build(debug=False):
    global _CACHED
    if _CACHED is not None and not debug:
        return _CACHED
    nc = bacc.Bacc("TRN2", target_bir_lowering=False, debug=False)

    def din(name, shape, dt=F32):
        return nc.dram_tensor(name, shape, dt, kind="ExternalInput").ap()

    map_rows_d = din("map_rows", [NS, E])
    mapT_d = din("mapT", [E, NS], BF16)
    obsT_d = din("obsT", [E, NO], BF16)
    wpb_d = din("wpb", [E, BW], BF16)
    wpf_d = din("wpf", [E, FW], F32R)
    c8_d = din("c8", [P, MPAD - H], FP8)
    vec_d = din("vpack", [3 * E + 1])
    out_d = nc.dram_tensor("out", [NS, E], F32, kind="ExternalOutput").ap()

    dbg = None
    if debug:
        def dout(name, shape, dt=F32):
            return nc.dram_tensor(name, shape, dt, kind="ExternalOutput").ap()
        dbg = {
            "qT": dout("dbg_qT", [P, NS], F32R),
            "gmT": dout("dbg_gmT", [H + 1, NS], F32R),
            "ags": dout("dbg_ags", [H + 1, NS], F32R),
            "okT": dout("dbg_okT", [P, NPAIR * P], F32R),
            "gob8": dout("dbg_gob8", [P, NPAIR * 2 * MPAD], FP8),
            "out_pre": dout("dbg_out_pre", [P, NT * E]),
            "mvC": dout("dbg_mvC", [P, NT * 2]),
        }

    with tile.TileContext(nc) as tc:
        _emit(tc, out_d, map_rows_d, mapT_d, obsT_d, wpb_d, wpf_d, c8_d,
              vec_d, dbg=dbg)
    nc.compile()
    if not debug:
        _CACHED = nc
    return nc


def _prep_in_maps(map_code, obs_code, Wq, Wk, Wv, Wo, bo, gamma, beta):
    f = np.float32
    map_code = np.ascontiguousarray(np.asarray(map_code, dtype=f))
    obs_code = np.asarray(obs_code, dtype=f)

    bf16_np = mybir.dt.np(BF16)
    fp8_np = mybir.dt.np(FP8)

    def to_bf16(x):
        return np.ascontiguousarray(np.asarray(x, dtype=f).astype(bf16_np))

    obsT = np.ascontiguousarray(obs_code.T)

    wq2 = np.concatenate([np.asarray(Wq, f), np.asarray(Wq, f)], axis=1)
    wk2 = np.concatenate([np.asarray(Wk, f), np.asarray(Wk, f)], axis=1)
    woe = np.zeros((E, E + 2), dtype=f)
    woe[0:H, 0:E] = np.asarray(Wo, dtype=f)
    woe[H, E] = 1.0
    wpb = np.zeros((E, BW), dtype=f)
    wpb[:, _WQ0:_WQ0 + 128] = wq2
    wpb[:, _WK0:_WK0 + 128] = wk2
    wpb[:, _WV0:_WV0 + E] = np.asarray(Wv, f)

    wpf = np.zeros((E, FW), dtype=f)
    wpf[:, _ONES0] = 1.0
    wpf[H, _SEL0 + H] = 1.0   # sel row: 1.0 at position 32 (partition 32)
    wpf[:, _WOE0:_WOE0 + E + 2] = woe

    c8 = np.zeros((P, MPAD - H), dtype=fp8_np)
    c8[:, 0] = 1.0

    vpack = np.concatenate([
        np.asarray(bo, dtype=f), np.asarray(gamma, dtype=f),
        np.asarray(beta, dtype=f), np.full((1,), -SHIFT, dtype=f),
    ])
    shared = {
        "obsT": to_bf16(obsT),
        "wpb": to_bf16(wpb),
        "wpf": np.ascontiguousarray(wpf),
        "c8": np.ascontiguousarray(c8),
        "vpack": np.ascontiguousarray(vpack),
    }
    in_maps = []
    for i in range(NCORES):
        shard = map_code[i * NS:(i + 1) * NS]
        m = dict(shared)
        m["map_rows"] = shard
        m["mapT"] = to_bf16(np.ascontiguousarray(shard.T))
        in_maps.append(m)
    return in_maps


def run(trace=False, **inputs):
    nc = _build()
    in_maps = _prep_in_maps(**inputs)
    res = run_bass_kernel_spmd(nc, in_maps, list(range(NCORES)), trace=trace)
    out = np.concatenate([res.results[i]["out"] for i in range(NCORES)], axis=0)
    return out, res


def kernel(**inputs):
    out, _ = run(trace=False, **inputs)
    return out


# revision 34
# speedup vs baseline: 1.3049x; 1.0166x over previous
"""Trainium2 Bass kernel for nn_Attention_5815385719367 (gnn_message_passing).

Computation (see reference):
  map_q/k/v = map_code @ Wq/Wk/Wv ; obs_k/v = obs_code @ Wk/Wv
  scores    = [sum(q*k,-1) | q @ obs_k.T] / 8
  w         = softmax(scores)
  agg       = w[:, :1]*glu(map_v) + w[:, 1:] @ glu(obs_v)
  out       = LN(agg @ Wo + bo + map_code) * gamma + beta

Sharding: data-parallel over N_map rows (2048 rows/core x 8 cores);
obs_code and weights replicated. No collectives.

v2 design notes (per core):
  - scores computed TRANSPOSED in PSUM: ST[obs=128, map] via PE ROW-TILED
    pairs: even obs block's k.T sits on SBUF partitions 0-63 (PE tile
    (0,0)), odd block's on partitions 64-127 (tile (64,0)); the two
    256-cycle streams run CONCURRENTLY in different PE row groups, so
    ST costs ~0.5 cyc/col. qT and okT are produced pre-duplicated /
    parity-split by projecting with host-duplicated weights
    ([64,128] wq|wq and wk|wk), so no cross-partition copies exist.
  - softmax exp is the hard wall (1 elem/lane/cycle on ACT): split it
    between ACT (direct exp -> fp8e4m3, logits shifted by -1 so
    exp <= ~110 < 240 = trn-e4m3 max) and DVE (Schraudolph: one
    mult-add tensor_scalar into uint8, whose bit pattern IS the
    e4m3 log-domain approximation; errors ~+-5% are noise-like and
    average out over 8k obs).
  - PV runs fp8 DoubleRow over block PAIRS: stationary
    gob8[128, 2, 66] = glu(obs_v)|ones|zero-pad for (even, odd)
    blocks, moving pt8[128, 2, 512] = exp'd scores; contraction is
    256 at 0.5 cyc/col. M padded 33->66 keeps col tiling off
    (DoubleRow is incompatible with column tiling). The ones column
    accumulates the softmax denominator for free.
  - self-attention term folded in after Wo (as v1): agg row 32 is
    seeded with selfexp, epilogue adds selfexp*(glu(map_v)@Wo) and
    divides by the denominator; the -1 logit shift cancels exactly.
  - projections in bf16 (inputs shipped bf16), epilogue Wo in bf16.
  - GPSIMD does SBUF-side elementwise work (sigmoid affine steps,
    map+bo, gamma/beta) since it cannot touch PSUM.
"""

import numpy as np

import concourse.bass as bass
import concourse.bacc as bacc
import concourse.tile as tile
from concourse import mybir
from concourse.bass_utils import run_bass_kernel_spmd

NCORES = 8
NM, NO, E = 16384, 8192, 64
NS = NM // NCORES            # 2048 map rows per core
H = E // 2                   # 32
TEMP = 8.0
EPS = 1e-6
P = 128
NT = NS // P                 # 16 row tiles per core
GW = 512                     # map group width (psum bank)
NPAIR = NO // 256            # 32 obs block-pairs
SHIFT = -2.0                 # logit shift: exp(l - SHIFT), cancels in ratio.
                             # Logits empirically span [-8.1, 8.32]; e5m2's
                             # 22-e-fold range with shift -2 covers all of it
                             # (max exp ~ e^10.3 = 3e4 < 57344) with no
                             # clipping at either end.
MPAD = 80                    # padded PV output partitions (33 real; %16 keeps the DoubleRow ldweights step legal, >64 keeps column tiling off)

F32 = mybir.dt.float32
F32R = mybir.dt.float32r
BF16 = mybir.dt.bfloat16
FP8 = mybir.dt.float8e4
FP8E5 = mybir.dt.float8e5
U8 = mybir.dt.uint8
AF = mybir.ActivationFunctionType
ALU = mybir.AluOpType
DR = mybir.MatmulPerfMode.DoubleRow

# Schraudolph constants for uint8 e5m2 log-domain exp of RAW score s:
#   i = 4*log2(exp(s/8 - SHIFT)) + 60 - sawtooth_center
# DVE float->uint8 conversion saturates [0,255] and rounds RNE (probed).
SCH_A = 4.0 * 1.4426950408889634 / TEMP   # 0.72135
SCH_B = 60.0 - 4.0 * 1.4426950408889634 * SHIFT - 0.229

# layout of the bf16 weight pack [64, BW]
_WQ0 = 0              # wq duplicated [64, 128]
_WK0 = 128            # wk duplicated [64, 128]
_WV0 = 256            # wv [64, 64]
BW = 320

# layout of the f32r pack [64, FW]
_ONES0 = 0            # ones column [64, 1]
_IDO = 1              # identity [33, MPAD] seed stationary
_WOE0 = 1 + MPAD      # woe [33 rows used, 66]
FW = 1 + MPAD + E + 2


def _bc_part(ap, n):
    """Broadcast a [x, ...] AP along a new leading partition dim of n."""
    return bass.AP(tensor=ap.tensor, offset=ap.offset, ap=[[0, n]] + list(ap.ap))


def _emit(tc, out_d, map_rows_d, mapT_d, obsT_d, wpb_d, wpf_d, c8_d, vec_d,
          dbg=None, exp_act_frac=0.5):
    nc = tc.nc
    with tc.tile_pool(name="consts", bufs=1) as consts, \
         tc.tile_pool(name="big", bufs=1) as big, \
         tc.tile_pool(name="sb_sm", bufs=3) as sb_sm, \
         tc.tile_pool(name="sb_pt", bufs=4) as sb_pt, \
         tc.tile_pool(name="ps", bufs=3, space="PSUM") as ps, \
         tc.tile_pool(name="ps_agg", bufs=2, space="PSUM") as ps_agg:

        # ---------------- constants ----------------
        wpb = consts.tile([E, BW], BF16)          # bf16 weights pack
        nc.sync.dma_start(wpb, wpb_d)
        wq2 = wpb[:, _WQ0:_WQ0 + 128]             # [64,128] wq|wq
        wk2 = wpb[:, _WK0:_WK0 + 128]             # [64,128] wk|wk
        wv = wpb[:, _WV0:_WV0 + E]                # [64,64]

        wpf = consts.tile([E, FW], F32R)
        nc.sync.dma_start(wpf, wpf_d)
        ones64 = wpf[:, _ONES0:_ONES0 + 1]
        id33 = wpf[0:H + 1, _IDO:_IDO + MPAD]     # identity seed [33, 80]
        woe = wpf[0:H + 1, _WOE0:_WOE0 + E + 2]   # [33,66]

        vecs = consts.tile([P, 3 * E + 1], F32)   # bo|gamma|beta|-shift
        nc.sync.dma_start(vecs, _bc_part(vec_d, P))
        bo_b = vecs[:, 0:E]
        ga_b = vecs[:, E:2 * E]
        be_b = vecs[:, 2 * E:3 * E]
        msh = vecs[:, 3 * E:3 * E + 1]            # -SHIFT bias column

        # ---------------- big arenas + input DMAs ----------------
        mapT = big.tile([E, NS], BF16)
        obsT = big.tile([E, NO], BF16)
        for lo, hi, t_, s_ in ((0, 512, mapT, mapT_d),
                               (0, 1024, obsT, obsT_d),
                               (512, 1024, mapT, mapT_d),
                               (1024, 2048, obsT, obsT_d),
                               (1024, 2048, mapT, mapT_d),
                               (2048, 4096, obsT, obsT_d),
                               (4096, 8192, obsT, obsT_d)):
            nc.sync.dma_start(t_[:, lo:hi], s_[:, lo:hi])
        map_rows = big.tile([P, NT, E], F32)
        nc.sync.dma_start(map_rows, map_rows_d.rearrange("(t p) e -> p t e", p=P))

        qT = big.tile([P, NS], F32R)              # map_q.T duplicated halves
        gmT = big.tile([H + 1, NS], F32R)         # [glu(map_v).T ; selfexp]
        okT = big.tile([P, NPAIR, P], F32R)       # obs_k.T parity-split
        gob8 = big.tile([P, NPAIR, 2, MPAD], FP8)  # glu(obs_v)|1|0 pairs
        ags = big.tile([H + 1, NS], F32R)         # [numer.T ; denom]
        map_pb = big.tile([P, NT, E], F32)        # map + bo
        out_pre = big.tile([P, NT, E], F32)
        out_all = big.tile([P, NT, E], F32)
        mvC = big.tile([P, NT, 2], F32)
        rstd = big.tile([P, NT], F32)

        # gob8 static columns: ones at h=32, zeros at h=33..65 (DMA from
        # the small HBM consts tensor, replicated via zero strides)
        gob8f = gob8.rearrange("p a b c -> p (a b) c")
        C8W = MPAD - H
        ones_src = bass.AP(tensor=c8_d.tensor, offset=c8_d.offset,
                           ap=[[C8W, P], [0, 2 * NPAIR], [0, 1]])
        nc.sync.dma_start(gob8f[:, :, H:H + 1], ones_src)
        zero_src = bass.AP(tensor=c8_d.tensor, offset=c8_d.offset + 1,
                           ap=[[C8W, P], [0, 2 * NPAIR], [1, MPAD - H - 1]])
        nc.sync.dma_start(gob8f[:, :, H + 1:MPAD], zero_src)

        # map + bo on gpsimd (all-SBUF)
        bo_rep = bass.AP(tensor=bo_b.tensor, offset=bo_b.offset,
                         ap=[list(bo_b.ap[0]), [0, NT], [1, E]])
        nc.gpsimd.tensor_tensor(out=map_pb, in0=map_rows, in1=bo_rep,
                                op=ALU.add)

        # ---------------- prologue pieces ----------------
        def map_chunk(c):
            """q (duplicated), selfexp, glu(map_v) for map cols [c*512, ..)."""
            sl = slice(c * GW, (c + 1) * GW)
            q_ps = ps.tile([P, 2, GW], F32, tag="st", name=f"qps{c}")
            nc.tensor.matmul(q_ps[:, 0, :], wq2, mapT[:, sl],
                             start=True, stop=True)
            nc.vector.tensor_copy(qT[:, sl], q_ps[:, 0, :])
            k_ps = ps.tile([P, 2, GW], F32, tag="st", name=f"kps{c}")
            nc.tensor.matmul(k_ps[:, 0, :], wk2, mapT[:, sl],
                             start=True, stop=True)
            qk = sb_sm.tile([E, GW], F32R, tag="qk", name=f"qk{c}")
            nc.vector.tensor_tensor(out=qk, in0=qT[0:E, sl],
                                    in1=k_ps[0:E, 0, :], op=ALU.mult)
            # self-score sum lands in the unused upper half of k_ps
            ss_ps = k_ps[0:1, 1, :]
            nc.tensor.matmul(ss_ps, ones64, qk, start=True, stop=True)
            nc.scalar.activation(gmT[H:H + 1, sl], ss_ps, AF.Exp,
                                 scale=1.0 / TEMP, bias=msh[0:1])
            v_ps = ps.tile([P, 2, GW], F32, tag="st", name=f"vps{c}")
            nc.tensor.matmul(v_ps[0:E, 0, :], wv, mapT[:, sl],
                             start=True, stop=True)
            th = sb_sm.tile([H, GW], F32, tag="th", name=f"th{c}")
            nc.scalar.activation(th, v_ps[H:E, 0, :], AF.Tanh, scale=0.5)
            nc.gpsimd.tensor_scalar(out=th, in0=th, scalar1=0.5, scalar2=0.5,
                                    op0=ALU.mult, op1=ALU.add)
            nc.vector.tensor_tensor(out=gmT[0:H, sl], in0=v_ps[0:H, 0, :],
                                    in1=th, op=ALU.mult)

        def obs_k_chunk2(c2, eng="v"):
            """okT parity-split fill for TWO obs chunks (one PSUM alloc).

            chunk c covers obs cols [c*512, ..) = blocks 4c..4c+3; even
            blocks land on partitions 0-63 of okT, odd blocks on 64-127
            (via the duplicated upper half of the wk2 projection, so no
            cross-partition movement is needed)."""
            k_ps = ps.tile([P, 2, GW], F32, tag="st", name=f"okps{c2}")
            for t in range(2):
                c = 2 * c2 + t
                sl = slice(c * GW, (c + 1) * GW)
                nc.tensor.matmul(k_ps[:, t, :], wk2, obsT[:, sl],
                                 start=True, stop=True)
            for t in range(2):
                c = 2 * c2 + t
                ev_in = k_ps[0:E, t, :].rearrange("p (b m) -> p b m",
                                                  b=2)[:, :, 0:P]
                ev_out = okT[0:E, 2 * c:2 * c + 2, :]
                od_in = k_ps[E:P, t, :].rearrange("p (b m) -> p b m",
                                                  b=2)[:, :, P:2 * P]
                od_out = okT[E:P, 2 * c:2 * c + 2, :]
                if eng == "v":
                    nc.vector.tensor_copy(ev_out, ev_in)
                    nc.vector.tensor_copy(od_out, od_in)
                else:
                    nc.scalar.copy(ev_out, ev_in)
                    nc.scalar.copy(od_out, od_in)

        def obs_v_batch2(c2, nb=16):
            """glu(obs_v) for nb consecutive obs blocks (one PSUM alloc)."""
            v_ps = ps.tile([P, 16, E], F32, tag="st", name=f"ovps{c2}")
            for b in range(nb):
                blk = c2 * 16 + b
                nc.tensor.matmul(v_ps[:, b, :],
                                 obsT[:, blk * P:(blk + 1) * P], wv,
                                 start=True, stop=True)
            tho = sb_sm.tile([P, 16, H], F32, tag="tho", name=f"tho{c2}")
            nc.scalar.activation(tho[:, 0:nb, :], v_ps[:, 0:nb, H:E],
                                 AF.Tanh, scale=0.5)
            nc.gpsimd.tensor_scalar(out=tho[:, 0:nb, :], in0=tho[:, 0:nb, :],
                                    scalar1=0.5, scalar2=0.5,
                                    op0=ALU.mult, op1=ALU.add)
            # blocks 16*c2.. -> pairs 8*c2.., t = parity
            og = gob8[:, 8 * c2:8 * c2 + nb // 2, :, 0:H]
            vi = v_ps[:, 0:nb, 0:H].rearrange("p (a b) h -> p a b h", b=2)
            ti = tho[:, 0:nb, :].rearrange("p (a b) h -> p a b h", b=2)
            nc.vector.tensor_tensor(out=og, in0=vi, in1=ti, op=ALU.mult)

        def agg_flush(g, agg, eng="v"):
            sl = slice(g * GW, (g + 1) * GW)
            if eng == "v":
                nc.vector.tensor_copy(ags[0:H + 1, sl], agg[0:H + 1, :])
            else:
                nc.scalar.copy(ags[0:H + 1, sl], agg[0:H + 1, :])

        # ---------------- epilogue ----------------
        def epi_half(half):
            """Batched epilogue for 8 map tiles: all PE matmuls first
            (into two grouped PSUM tiles, 512B-strided so every [128,66]
            output stays within one bank), then the elementwise chain
            pipelines across tiles on DVE/ACT without PE round-trips."""
            base = half * (NT // 2)
            uda = ps.tile([P, 8, P], F32, tag="st", name=f"uda{half}")
            for i in range(8):
                sl = slice((base + i) * P, (base + i + 1) * P)
                nc.tensor.matmul(uda[:, i, 0:E + 2], ags[:, sl], woe,
                                 start=True, stop=True)
            # evacuate PSUM immediately (uda sits in the ST rotation --
            # holding it through the elementwise chain would strangle the
            # main loop when this runs inside hp1)
            uds = sb_sm.tile([P, 8, E + 2], F32, tag="uds", name=f"uds{half}")
            nc.vector.tensor_copy(uds, uda[:, :, 0:E + 2])
            rden = sb_sm.tile([P, 8], F32, tag="rden", name=f"rden{half}")
            nc.vector.reciprocal(rden, uds[:, :, E])
            for i in range(8):
                t = base + i
                nc.vector.scalar_tensor_tensor(out=out_pre[:, t, :],
                                               in0=uds[:, i, 0:E],
                                               scalar=rden[:, i:i + 1],
                                               in1=map_pb[:, t, :],
                                               op0=ALU.mult, op1=ALU.add)
                stats = sb_sm.tile([P, 6], F32, tag="stats", name=f"stats{t}")
                nc.vector.bn_stats(stats, out_pre[:, t, :])
                nc.vector.bn_aggr(mvC[:, t, :], stats)

        def epi_final(half, act_assist=True):
            tsl = slice(half * (NT // 2), (half + 1) * (NT // 2))
            w = NT // 2
            vpe = sb_sm.tile([P, w], F32, tag="vpe", name=f"vpe{half}")
            nc.vector.tensor_scalar_add(vpe, mvC[:, tsl, 1], EPS)
            c1 = sb_sm.tile([P, w], F32, tag="nc1", name=f"nc1{half}")
            nc.vector.tensor_scalar(out=c1, in0=vpe, scalar1=0.564185,
                                    scalar2=0.378467, op0=ALU.mult,
                                    op1=ALU.add)
            c2 = sb_sm.tile([P, w], F32, tag="nc2", name=f"nc2{half}")
            nc.vector.tensor_scalar(out=c2, in0=vpe, scalar1=0.288949,
                                    scalar2=0.791321, op0=ALU.mult,
                                    op1=ALU.add)
            nc.vector.tensor_tensor(out=c1, in0=c1, in1=c2, op=ALU.min)
            rs = rstd[:, tsl]
            nc.vector.reciprocal(rs, c1)
            for _ in range(3):
                nc.vector.tensor_tensor(out=c1, in0=rs, in1=rs, op=ALU.mult)
                nc.vector.tensor_tensor(out=c1, in0=c1, in1=vpe, op=ALU.mult)
                nc.vector.tensor_scalar(out=c1, in0=c1, scalar1=-0.5,
                                        scalar2=1.5, op0=ALU.mult,
                                        op1=ALU.add)
                nc.vector.tensor_tensor(out=rs, in0=rs, in1=c1, op=ALU.mult)
            for t in range(half * (NT // 2), (half + 1) * (NT // 2)):
                xn = sb_sm.tile([P, E], F32, tag="xn", name=f"xn{t}")
                if act_assist:
                    nmr = sb_sm.tile([P, 1], F32, tag="nmr", name=f"nmr{t}")
                    nc.vector.tensor_scalar(out=nmr, in0=mvC[:, t, 0:1],
                                            scalar1=rstd[:, t:t + 1],
                                            scalar2=-1.0, op0=ALU.mult,
                                            op1=ALU.mult)
                    nc.scalar.activation(xn, out_pre[:, t, :], AF.Identity,
                                         bias=nmr, scale=rstd[:, t:t + 1])
                else:
                    nc.vector.tensor_scalar(out=xn, in0=out_pre[:, t, :],
                                            scalar1=mvC[:, t, 0:1],
                                            scalar2=rstd[:, t:t + 1],
                                            op0=ALU.subtract, op1=ALU.mult)
                nc.gpsimd.tensor_tensor(out=xn, in0=xn, in1=ga_b, op=ALU.mult)
                nc.gpsimd.tensor_tensor(out=out_all[:, t, :], in0=xn,
                                        in1=be_b, op=ALU.add)
            od = out_d.rearrange("(t p) e -> p t e", p=P)
            for q in range(2):
                qsl = slice(half * (NT // 2) + q * (NT // 4),
                            half * (NT // 2) + (q + 1) * (NT // 4))
                nc.sync.dma_start(od[:, qsl, :], out_all[:, qsl, :])

        # ---------------- prologue head ----------------
        map_chunk(0)
        map_chunk(1)
        obs_k_chunk2(0)
        obs_v_batch2(0)
        map_chunk(2)
        map_chunk(3)
        # replicate the selfexp row across 32 partitions (via an HBM
        # bounce: neither engines nor SBUF-source DMAs can partition-
        # broadcast, but a DRAM-source DMA can). Then fold the self term
        # into gmT: rows 0-31 become selfexp * glu(map_v).T, which the
        # identity seed matmul deposits directly into the agg accumulators.
        sxp_hbm = nc.dram_tensor("sxp_hbm", [NS], F32R, kind="Internal").ap()
        nc.sync.dma_start(sxp_hbm, gmT[H:H + 1, :])
        sxp = big.tile([H, NS], F32R)
        nc.sync.dma_start(sxp, _bc_part(sxp_hbm, H))
        nc.vector.tensor_tensor(out=gmT[0:H, :], in0=gmT[0:H, :],
                                in1=sxp, op=ALU.mult)

        # drip the remaining prologue into the first half-pass
        # drip schedule. IMPORTANT: obs_v_batch stays a single drip unit --
        # its PSUM tile comes from the shared rotating "st" tag, so the glu
        # must read it before the main loop's next st allocations wrap
        # around the pool and clobber the bank.
        drip = {}
        items = []
        for c2 in range(1, NO // GW // 2):
            items.append((4 * (c2 - 1), lambda c2=c2: obs_k_chunk2(c2)))
        for b2 in range(1, 4):
            items.append((7 * b2 - 3, lambda b2=b2: obs_v_batch2(b2)))
        items.sort(key=lambda x: x[0])
        used = set()
        for want, fn in items:
            pp = want
            while pp in used:
                pp += 1
            used.add(pp)
            drip.setdefault(pp, []).append(fn)

        # exp unit assignment: alternate engines per (pair, group); bias
        # toward ACT by granting it both groups every few pairs.
        def exp_unit(st_t, pt_t, eng):
            if eng == "a":
                nc.scalar.activation(pt_t, st_t, AF.Exp,
                                     scale=1.0 / TEMP, bias=msh)
            else:
                nc.vector.tensor_scalar(out=pt_t.bitcast(U8), in0=st_t,
                                        scalar1=SCH_A, scalar2=SCH_B,
                                        op0=ALU.mult, op1=ALU.add)

        # ---------------- main loop: 2 half-passes x 32 pairs ----------
        # Software-pipelined by one pair: the PV for pair p-1 is issued to
        # the PE AFTER pair p's ST matmuls, so by the time the PE FIFO
        # reaches it, exp(p-1) has long finished -- no head-of-line stall.
        for hp in range(2):
            agg0 = ps_agg.tile([MPAD, GW], F32, tag="agg", name=f"agg{hp}_0")
            agg1 = ps_agg.tile([MPAD, GW], F32, tag="agg", name=f"agg{hp}_1")
            g0 = 2 * hp
            g1 = 2 * hp + 1
            s0 = slice(g0 * GW, (g0 + 1) * GW)
            s1 = slice(g1 * GW, (g1 + 1) * GW)
            nc.tensor.matmul(agg0, id33, gmT[:, s0],
                             start=True, stop=False)
            nc.tensor.matmul(agg1, id33, gmT[:, s1],
                             start=True, stop=False)
            prev_pt = None
            for pp in range(NPAIR):
                st0 = ps.tile([P, 2, GW], F32, tag="st", name=f"st{hp}_{pp}_0")
                st1 = ps.tile([P, 2, GW], F32, tag="st", name=f"st{hp}_{pp}_1")
                ko_lo = okT[0:E, pp, :]
                ko_hi = okT[E:P, pp, :]
                nc.tensor.matmul(st0[:, 0, :], ko_lo, qT[0:E, s0],
                                 start=True, stop=True)
                nc.tensor.matmul(st0[:, 1, :], ko_hi, qT[E:P, s0],
                                 start=True, stop=True)
                nc.tensor.matmul(st1[:, 0, :], ko_lo, qT[0:E, s1],
                                 start=True, stop=True)
                nc.tensor.matmul(st1[:, 1, :], ko_hi, qT[E:P, s1],
                                 start=True, stop=True)
                if prev_pt is not None:
                    qq, qt0, qt1 = prev_pt
                    go = gob8[:, qq, :, :]
                    nc.tensor.matmul(agg0, go, qt0, start=False, stop=False,
                                     perf_mode=DR)
                    nc.tensor.matmul(agg1, go, qt1, start=False, stop=False,
                                     perf_mode=DR)
                pt0 = sb_pt.tile([P, 2, GW], FP8E5, tag="pt",
                                 name=f"pt{hp}_{pp}_0")
                pt1 = sb_pt.tile([P, 2, GW], FP8E5, tag="pt",
                                 name=f"pt{hp}_{pp}_1")
                # exp split: in hp0 DVE also carries the drip (casts/glu),
                # so ACT takes both groups every 4th pair; in hp1 the
                # engines are evenly loaded, so strict 1:1.
                bonus = (pp % 4 == 3) if hp == 0 else False
                exp_unit(st0, pt0, "a")
                exp_unit(st1, pt1, "a" if bonus else "v")
                prev_pt = (pp, pt0, pt1)
                if hp == 0:
                    for fn in drip.get(pp, ()):
                        fn()
                else:
                    # hp0's ags columns are final: run its epilogue during
                    # hp1 (batched -- only two extra PSUM allocs total)
                    if pp == 6:
                        epi_half(0)
                    elif pp == 16:
                        epi_final(0)
            qq, qt0, qt1 = prev_pt
            go = gob8[:, qq, :, :]
            nc.tensor.matmul(agg0, go, qt0, start=False, stop=True,
                             perf_mode=DR)
            nc.tensor.matmul(agg1, go, qt1, start=False, stop=True,
                             perf_mode=DR)
            agg_flush(g0, agg0, eng="v")
            agg_flush(g1, agg1, eng="a")

        # ---------------- epilogue (half 0 ran during hp1) ----------
        epi_half(1)
        epi_final(1)

        if dbg is not None:
            nc.sync.dma_start(dbg["qT"], qT)
            nc.sync.dma_start(dbg["gmT"], gmT)
            nc.sync.dma_start(dbg["ags"], ags)
            nc.sync.dma_start(dbg["okT"], okT.rearrange("p a b -> p (a b)"))
            nc.sync.dma_start(dbg["gob8"],
                              gob8.rearrange("p a b c -> p (a b c)"))
            nc.sync.dma_start(dbg["out_pre"],
                              out_pre.rearrange("p a b -> p (a b)"))
            nc.sync.dma_start(dbg["mvC"], mvC.rearrange("p a b -> p (a b)"))


_CACHED = None


def _build(debug=False):
    global _CACHED
    if _CACHED is not None and not debug:
        return _CACHED
    nc = bacc.Bacc("TRN2", target_bir_lowering=False, debug=False)

    def din(name, shape, dt=F32):
        return nc.dram_tensor(name, shape, dt, kind="ExternalInput").ap()

    map_rows_d = din("map_rows", [NS, E])
    mapT_d = din("mapT", [E, NS], BF16)
    obsT_d = din("obsT", [E, NO], BF16)
    wpb_d = din("wpb", [E, BW], BF16)
    wpf_d = din("wpf", [E, FW], F32R)
    c8_d = din("c8", [P, MPAD - H], FP8)
    vec_d = din("vpack", [3 * E + 1])
    out_d = nc.dram_tensor("out", [NS, E], F32, kind="ExternalOutput").ap()

    dbg = None
    if debug:
        def dout(name, shape, dt=F32):
            return nc.dram_tensor(name, shape, dt, kind="ExternalOutput").ap()
        dbg = {
            "qT": dout("dbg_qT", [P, NS], F32R),
            "gmT": dout("dbg_gmT", [H + 1, NS], F32R),
            "ags": dout("dbg_ags", [H + 1, NS], F32R),
            "okT": dout("dbg_okT", [P, NPAIR * P], F32R),
            "gob8": dout("dbg_gob8", [P, NPAIR * 2 * MPAD], FP8),
            "out_pre": dout("dbg_out_pre", [P, NT * E]),
            "mvC": dout("dbg_mvC", [P, NT * 2]),
        }

    with tile.TileContext(nc) as tc:
        _emit(tc, out_d, map_rows_d, mapT_d, obsT_d, wpb_d, wpf_d, c8_d,
              vec_d, dbg=dbg)
    nc.compile()
    if not debug:
        _CACHED = nc
    return nc


def _prep_in_maps(map_code, obs_code, Wq, Wk, Wv, Wo, bo, gamma, beta):
    f = np.float32
    map_code = np.ascontiguousarray(np.asarray(map_code, dtype=f))
    obs_code = np.asarray(obs_code, dtype=f)

    bf16_np = mybir.dt.np(BF16)
    fp8_np = mybir.dt.np(FP8)

    def to_bf16(x):
        return np.ascontiguousarray(np.asarray(x, dtype=f).astype(bf16_np))

    obsT = np.ascontiguousarray(obs_code.T)

    wq2 = np.concatenate([np.asarray(Wq, f), np.asarray(Wq, f)], axis=1)
    wk2 = np.concatenate([np.asarray(Wk, f), np.asarray(Wk, f)], axis=1)
    woe = np.zeros((E, E + 2), dtype=f)
    woe[0:H, 0:E] = np.asarray(Wo, dtype=f)
    woe[H, E] = 1.0
    wpb = np.zeros((E, BW), dtype=f)
    wpb[:, _WQ0:_WQ0 + 128] = wq2
    wpb[:, _WK0:_WK0 + 128] = wk2
    wpb[:, _WV0:_WV0 + E] = np.asarray(Wv, f)

    wpf = np.zeros((E, FW), dtype=f)
    wpf[:, _ONES0] = 1.0
    for k in range(H + 1):
        wpf[k, _IDO + k] = 1.0   # identity seed stationary [33, MPAD]
    wpf[:, _WOE0:_WOE0 + E + 2] = woe

    c8 = np.zeros((P, MPAD - H), dtype=fp8_np)
    c8[:, 0] = 1.0

    vpack = np.concatenate([
        np.asarray(bo, dtype=f), np.asarray(gamma, dtype=f),
        np.asarray(beta, dtype=f), np.full((1,), -SHIFT, dtype=f),
    ])
    shared = {
        "obsT": to_bf16(obsT),
        "wpb": to_bf16(wpb),
        "wpf": np.ascontiguousarray(wpf),
        "c8": np.ascontiguousarray(c8),
        "vpack": np.ascontiguousarray(vpack),
    }
    in_maps = []
    for i in range(NCORES):
        shard = map_code[i * NS:(i + 1) * NS]
        m = dict(shared)
        m["map_rows"] = shard
        m["mapT"] = to_bf16(np.ascontiguousarray(shard.T))
        in_maps.append(m)
    return in_maps


def run(trace=False, **inputs):
    nc = _build()
    in_maps = _prep_in_maps(**inputs)
    res = run_bass_kernel_spmd(nc, in_maps, list(range(NCORES)), trace=trace)
    out = np.concatenate([res.results[i]["out"] for i in range(NCORES)], axis=0)
    return out, res


def kernel(**inputs):
    out, _ = run(trace=False, **inputs)
    return out
